# revision 5
# baseline (speedup 1.0000x reference)
"""GroupedQueryAttention Trainium2 kernel (8-core SPMD), bf16.

Problem: B=2, S=2048, D=2048, 32 Q heads, 8 KV groups, head_dim=64.
  q = xq @ Wq + bq; k = xk @ Wk + bk; v = xv @ Wv + bv
  logits = q . k / sqrt(512), causal softmax, out = (attn @ v) @ Wo + bo

Sharding: one batch x two KV groups per core (2 batches x 4 group-pairs = 8).
Each core computes its 8 Q heads' attention and a partial output projection;
the host sums the 4 partials per batch and adds the bv/bo corrections (exact
because attention rows sum to 1).

Device-side design (sim ~0.30 ms/core vs 1.42 ms for the fp32 baseline):
- All matmul operands bf16 (1 PE cycle/row vs 4 for fp32); PSUM stays fp32.
  Host ships x/W inputs pre-transposed and bf16 (halves DMA bytes).
- Logits computed transposed (lT[n, m]) so attn@v needs no transpose and
  softmax denominators ride the attn@v matmuls: each half's v weights carry
  a 65th all-ones column ([v | 1]), landing sum_n p[n, m] in psum row 64.
  Normalization: DVE reciprocal -> PE outer-product broadcast -> fused
  psum-eviction multiply (half-b shifts partitions 0:64 -> 64:128).
- One Exp activation per n-block over a 2-bank [128, 1024] psum tile (both
  group halves, strided AP on diagonal tiles); causal masking by skipping
  n>m blocks, trimming diagonal widths, one strided triangle-mask multiply.
- Batched DMA (~45 transfers vs 327: HWDGE costs ~630 ns per instruction),
  first K/Q blocks split so the PE starts ~5 us in, out rows stored bf16.
- Software pipelining: projections stream just-in-time; K/V/Q blocks and
  the previous superblock's Wo groups are emitted as single-psum-bank
  "filler" units between attention j-steps, so the PE has backlog while
  the attnv chain waits on the scalar engine's exp.
- Bias adds on the DVE (tensor_scalar) to keep the scalar engine free for
  the exp stream; psum banks: pj(proj/Wo/bcast) 2 + logits 4 + acc 2 = 8.
"""

import math
import numpy as np
import ml_dtypes

import concourse.bass as bass
import concourse.mybir as mybir
from concourse import tile
from concourse.bass_utils import run_bass_kernel_spmd
from concourse.vector_clock import ScopedClock

F32 = mybir.dt.float32
BF16 = mybir.dt.bfloat16
NPBF16 = ml_dtypes.bfloat16
B, S, D = 2, 2048, 2048
NKV, HPG, HD = 8, 4, 64
DIMK = 512
CPC = 512                  # q channels per core (2 groups * 4 heads * 64)
KC = D // 128              # 16 k-chunks
MSB = S // 512             # 4 m-superblocks
NB = S // 128              # 16 n-blocks
VW = 130                   # v_sb cols per n-block: 64 va | 1 | 64 vb | 1
                           # (each half's weights end with an all-ones col,
                           # so both denominators ride the attn@v matmuls
                           # into psum row 64 of their acc bank)
INV_SQRT_DIMK = 1.0 / math.sqrt(float(DIMK))


# ---------------------------------------------------------------------------
# TileContext tail-drain patch: the bundled neuronxcc walrus rejects
# instructions carrying more than ~2 sync waits ("Too many sync wait
# commands"). Spread the kernel-tail waits over single-wait nops.
def _patched_drain_and_barrier(self, tick_clock, wait_clock):
    nc = self.nc
    collector = nc.sync.nop(nofuse=True)
    wait_clock.add_sem_waits(
        collector.ins, ScopedClock({None: tick_clock.global_clock})
    )
    si = collector.ins.sync_info
    waits = list(si.on_wait) if si is not None and si.on_wait else []
    if waits:
        collector.ins.sync_info = mybir.SyncInfo(
            on_wait=[waits[0]], on_update=list(si.on_update or [])
        )
        for w in waits[1:]:
            extra = nc.sync.nop(nofuse=True)
            extra.ins.sync_info = mybir.SyncInfo(on_wait=[w], on_update=[])
    nc.sync.drain()
    nc.all_engine_barrier()
    assert self.sems is not None
    popped = nc._tile_sem_poison_stack.pop()
    assert popped is self._sem_poison
    nc.clear_and_free_semaphores(list(self.sems.allocated().values()))
    nc.all_engine_barrier()


tile.TileContext._drain_and_barrier = _patched_drain_and_barrier


_MAXW = 1
_NOPID = [0]


def split_excess_waits(nc):
    """Walrus here encodes at most ~1-2 sync waits per instruction; move the
    excess onto preceding same-engine nops (engine order preserves timing)."""
    for f in nc.m.functions:
        for bb in f.blocks:
            out_list = []
            changed = False
            for inst in bb.instructions:
                si = getattr(inst, "sync_info", None)
                waits = list(si.on_wait) if si is not None and si.on_wait else []
                if len(waits) > _MAXW:
                    changed = True
                    for w in waits[:-_MAXW]:
                        _NOPID[0] += 1
                        nop = mybir.InstNoOp(
                            name=f"waitnop-{_NOPID[0]}", ins=[], outs=[],
                            engine=inst.engine,
                        )
                        nop.sync_info = mybir.SyncInfo(on_wait=[w], on_update=[])
                        out_list.append(nop)
                    inst.sync_info = mybir.SyncInfo(
                        on_wait=waits[-_MAXW:], on_update=list(si.on_update or [])
                    )
                out_list.append(inst)
            if changed:
                bb.instructions[:] = out_list
# ---------------------------------------------------------------------------


def build_bass():
    nc = bass.Bass()
    xqT = nc.dram_tensor("xqT", [D, S], BF16, kind="ExternalInput")
    xkT = nc.dram_tensor("xkT", [D, S], BF16, kind="ExternalInput")
    xvT = nc.dram_tensor("xvT", [D, S], BF16, kind="ExternalInput")
    wq = nc.dram_tensor("wq", [D, CPC], BF16, kind="ExternalInput")
    wk = nc.dram_tensor("wk", [D, 128], BF16, kind="ExternalInput")
    wv = nc.dram_tensor("wv", [D, 128], BF16, kind="ExternalInput")
    wo = nc.dram_tensor("wo", [CPC, D], BF16, kind="ExternalInput")
    bq = nc.dram_tensor("bq", [CPC, 1], F32, kind="ExternalInput")
    bk = nc.dram_tensor("bk", [128, 1], F32, kind="ExternalInput")
    trimask = nc.dram_tensor("trimask", [128, 256], BF16, kind="ExternalInput")
    out = nc.dram_tensor("out", [S, D], BF16, kind="ExternalOutput")

    from contextlib import ExitStack
    with tile.TileContext(nc) as tc, ExitStack() as ctx:
        build_body(ctx, tc, xqT, xkT, xvT, wq, wk, wv, wo, bq, bk, trimask, out)
    split_excess_waits(nc)
    return nc


def build_body(ctx, tc, xqT, xkT, xvT, wq, wk, wv, wo, bq, bk, trimask, out):
    nc = tc.nc
    Exp = mybir.ActivationFunctionType.Exp
    Ident = mybir.ActivationFunctionType.Identity

    const = ctx.enter_context(tc.tile_pool(name="const", bufs=1))
    wq_sb = const.tile([128, KC * CPC], BF16, tag="wq")        # [128, 8192]
    wk_sb = const.tile([128, KC * 128], BF16, tag="wk")        # [128, 2048]
    wv_sb = const.tile([128, KC * 128], BF16, tag="wv")        # [128, 2048]
    wo_sb = const.tile([128, 4 * D], BF16, tag="wo")           # [128, 8192]
    kT_sb = const.tile([128, S], BF16, tag="kT")               # [128, 2048]
    v_sb = const.tile([128, NB * VW], BF16, tag="v")           # [128, 2080]
    qT_sb = const.tile([128, 4 * S], BF16, tag="qT")           # [128, 8192]
    bq_sb = const.tile([128, 4], F32, tag="bq")
    bk_sb = const.tile([128, 1], F32, tag="bk")
    mask_sb = const.tile([128, 256], BF16, tag="mask")
    ones_row = const.tile([1, 64], BF16, tag="ones_row")

    # Weight / bias / mask loads — ordered by first use (wk gates the K
    # projection at t=0; wq/wo aren't needed until ~60/~90us in) so the
    # xk stream isn't stuck behind 4MB of late-use weights.
    nc.sync.dma_start(bk_sb[:], bk[:])
    nc.sync.dma_start(mask_sb[:], trimask[:])
    nc.vector.memset(ones_row[:], 1.0)
    # all-ones columns interleaved in v_sb (denominators ride attn@v)
    for blk in range(NB):
        nc.vector.memset(v_sb[:, blk * VW + 64: blk * VW + 65], 1.0)
        nc.vector.memset(v_sb[:, blk * VW + 129: blk * VW + 130], 1.0)

    # psum budget: pj (K/V/Q proj + Wo + bcast) 2 + lt 4 + acc 2 = 8
    with tc.tile_pool(name="pj_psum", bufs=2, space="PSUM") as pj_psum, \
         tc.tile_pool(name="lt_psum", bufs=2, space="PSUM") as lt_psum, \
         tc.tile_pool(name="acc_psum", bufs=2, space="PSUM") as acc_psum, \
         tc.tile_pool(name="xin", bufs=2) as xin_pool, \
         tc.tile_pool(name="xvin", bufs=3) as xvin_pool, \
         tc.tile_pool(name="pt", bufs=4) as pt_pool, \
         tc.tile_pool(name="outT", bufs=2) as outT_pool, \
         tc.tile_pool(name="nrm", bufs=4) as nrm_pool, \
         tc.tile_pool(name="osb", bufs=2) as out_pool:

        xq_tiles = {}

        _qps_box = {}

        def q_proj_unit(msb, cb, u):
            if u == 0:
                _qps_box[(msb, cb)] = pj_psum.tile(
                    [128, 512], F32, tag="ps", name=f"psq{msb}{cb}")
            ps = _qps_box[(msb, cb)]
            xq_t = xq_tiles[msb]
            for kc in range(4 * u, 4 * u + 4):
                nc.tensor.matmul(
                    ps[:],
                    wq_sb[:, kc * CPC + cb * 128: kc * CPC + (cb + 1) * 128],
                    xq_t[:, kc * 512:(kc + 1) * 512],
                    start=(kc == 0), stop=(kc == KC - 1),
                )
            if u == 3:
                nc.vector.tensor_scalar_add(
                    qT_sb[:, cb * S + msb * 512: cb * S + (msb + 1) * 512],
                    ps[:], bq_sb[:, cb:cb + 1],
                )

        def q_proj_cb(msb, cb):
            for u in range(4):
                q_proj_unit(msb, cb, u)

        # --- single-bank filler units for projections -------------------
        # Each owner (a K block, a V half-pass, a Q cb-pass) accumulates in
        # one pj bank across its consecutive units, so units from different
        # owners can interleave with attention j-steps without deadlocking
        # the 2-bank pj ring.
        _kps_box = {}
        xk_tiles = {}

        def k_proj_unit(nsb, u):
            if u == 0:
                _kps_box[nsb] = pj_psum.tile([128, 512], F32, tag="ps",
                                             name=f"psk{nsb}")
            ps = _kps_box[nsb]
            xk_t = xk_tiles[nsb]
            for kc in range(4 * u, 4 * u + 4):
                nc.tensor.matmul(
                    ps[:], wk_sb[:, kc * 128:(kc + 1) * 128],
                    xk_t[:, kc * 512:(kc + 1) * 512],
                    start=(kc == 0), stop=(kc == KC - 1),
                )
            if u == 3:
                nc.vector.tensor_scalar_add(
                    kT_sb[:, nsb * 512:(nsb + 1) * 512], ps[:], bk_sb[:]
                )

        _vps_box = {}
        xv_tiles = {}

        def v_proj_unit(nbp, half, u):
            if u == 0:
                _vps_box[(nbp, half)] = pj_psum.tile(
                    [128, 128], F32, tag="ps", name=f"psv{nbp}{half}")
            ps = _vps_box[(nbp, half)]
            xv_t = xv_tiles[nbp]
            for kc in range(4 * u, 4 * u + 4):
                nc.tensor.matmul(
                    ps[:], xv_t[:, kc * 256 + half * 128:
                                kc * 256 + half * 128 + 128],
                    wv_sb[:, kc * 128:(kc + 1) * 128],
                    start=(kc == 0), stop=(kc == KC - 1),
                )
            if u == 3:
                blk = 2 * nbp + half
                nc.vector.tensor_copy(
                    v_sb[:, blk * VW: blk * VW + 64], ps[:, 0:64])
                nc.vector.tensor_copy(
                    v_sb[:, blk * VW + 65: blk * VW + 129], ps[:, 64:128])

        def xk_dma(nsb):
            xk_t = xin_pool.tile([128, KC * 512], BF16, tag="xk",
                                 name=f"xk{nsb}")
            nc.sync.dma_start(
                xk_t[:].rearrange("p (kc n) -> p kc n", kc=KC),
                xkT.rearrange("(kc p) n -> p kc n", p=128)[
                    :, :, nsb * 512:(nsb + 1) * 512],
            )
            xk_tiles[nsb] = xk_t

        def xv_dma(nbp):
            xv_t = xvin_pool.tile([128, KC * 256], BF16, tag="xv",
                                  name=f"xv{nbp}")
            nc.sync.dma_start(
                xv_t[:].rearrange("p (kc n) -> p kc n", kc=KC),
                xvT.rearrange("(kc p) n -> p kc n", p=128)[
                    :, :, nbp * 256:(nbp + 1) * 256],
            )
            xv_tiles[nbp] = xv_t

        # --- phase A: the minimum needed before attention msb0 ----------
        # K block 0 (4 sub-DMAs so the PE starts ~5us in), V blocks 0:4,
        # Q msb0 — everything else streams just-in-time as filler work.
        def wk_dma(h):
            nc.sync.dma_start(
                wk_sb[:, h * 4 * 128:(h + 1) * 4 * 128].rearrange(
                    "p (kc c) -> p kc c", kc=4),
                wk.rearrange("(kc p) c -> p kc c", p=128)[
                    :, h * 4:(h + 1) * 4],
            )

        with tc.tile_pool(name="xk0", bufs=4) as xk0_pool:
            xk0_t = [xk0_pool.tile([128, 4 * 512], BF16, tag="xk0",
                                   name=f"xk0{i}") for i in range(4)]
            for i in range(4):
                wk_dma(i)
                nc.sync.dma_start(
                    xk0_t[i][:].rearrange("p (kc n) -> p kc n", kc=4),
                    xkT.rearrange("(kc p) n -> p kc n", p=128)[
                        :, 4 * i: 4 * i + 4, 0:512],
                )
            nc.sync.dma_start(
                wv_sb[:].rearrange("p (kc c) -> p kc c", kc=KC),
                wv.rearrange("(kc p) c -> p kc c", p=128),
            )
            nc.sync.dma_start(
                bq_sb[:].rearrange("p (cb o) -> p cb o", cb=4),
                bq.rearrange("(cb p) o -> p cb o", p=128),
            )
            # xv0 ahead of the wq/xq0 halves: V0 is the first PE work
            # after K0; the Q chunks land while V0 runs.
            xv_dma(0)
            xq_t0 = xin_pool.tile([128, KC * 512], BF16, tag="xk", name="xq0")
            xq_tiles[0] = xq_t0
            for h in range(2):
                nc.sync.dma_start(
                    xq_t0[:, h * 8 * 512:(h + 1) * 8 * 512].rearrange(
                        "p (kc n) -> p kc n", kc=8),
                    xqT.rearrange("(kc p) n -> p kc n", p=128)[
                        :, h * 8:(h + 1) * 8, 0:512],
                )
                nc.sync.dma_start(
                    wq_sb[:, h * 8 * CPC:(h + 1) * 8 * CPC].rearrange(
                        "p (kc c) -> p kc c", kc=8),
                    wq.rearrange("(kc p) c -> p kc c", p=128)[
                        :, h * 8:(h + 1) * 8],
                )
                if h == 0:
                    xv_dma(1)
            ps = pj_psum.tile([128, 512], F32, tag="ps", name="psk0")
            for kc in range(KC):
                nc.tensor.matmul(
                    ps[:], wk_sb[:, kc * 128:(kc + 1) * 128],
                    xk0_t[kc // 4][:, (kc % 4) * 512:(kc % 4 + 1) * 512],
                    start=(kc == 0), stop=(kc == KC - 1),
                )
            nc.vector.tensor_scalar_add(kT_sb[:, 0:512], ps[:], bk_sb[:])

        for half in range(2):
            for u in range(4):
                v_proj_unit(0, half, u)
        q_proj_cb(0, 0)
        for half in range(2):
            for u in range(4):
                v_proj_unit(1, half, u)
        q_proj_cb(0, 1)
        q_proj_cb(0, 2)
        q_proj_cb(0, 3)

        # Wo for one m-superblock as 16 filler units (one per mb/db pso
        # group); the out-row DMA rides the db==3 unit.
        def wo_group(msb, outT_t, o_t_box, mb, db):
            if db == 0:
                o_t_box[mb] = out_pool.tile([128, 2048], BF16, tag="osb",
                                            name=f"osb{msb}{mb}")
            o_t = o_t_box[mb]
            pso = pj_psum.tile([128, 512], F32, tag="ps",
                               name=f"pso{msb}{mb}{db}")
            for cb in range(4):
                nc.tensor.matmul(
                    pso[:],
                    outT_t[:, cb * 512 + mb * 128: cb * 512 + (mb + 1) * 128],
                    wo_sb[:, cb * D + db * 512: cb * D + (db + 1) * 512],
                    start=(cb == 0), stop=(cb == 3),
                )
            nc.vector.tensor_copy(o_t[:, db * 512:(db + 1) * 512], pso[:])
            if db == 3:
                nc.sync.dma_start(
                    out[msb * 512 + mb * 128: msb * 512 + (mb + 1) * 128, :],
                    o_t[:],
                )

        # Per m-superblock: attention with filler work (Wo groups of the
        # previous superblock + Q-projection passes of the next) emitted
        # between j-steps so the PE has backlog whenever the attnv chain
        # waits on the scalar engine's exp.
        pending = []   # filler closures, emitted oldest-first
        for msb in range(MSB):
            if msb + 1 < MSB:
                xk_dma(msb + 1)
                xv_dma(2 * msb + 2)
                xv_dma(2 * msb + 3)
                if msb == 0:
                    nc.sync.dma_start(
                        wo_sb[:].rearrange("p (cb d) -> p cb d", cb=4),
                        wo.rearrange("(cb p) d -> p cb d", p=128),
                    )
                xq_t = xin_pool.tile([128, KC * 512], BF16, tag="xk",
                                     name=f"xq{msb + 1}")
                nc.sync.dma_start(
                    xq_t[:].rearrange("p (kc n) -> p kc n", kc=KC),
                    xqT.rearrange("(kc p) n -> p kc n", p=128)[
                        :, :, (msb + 1) * 512:(msb + 2) * 512],
                )
                xq_tiles[msb + 1] = xq_t
                for u in range(4):
                    pending.append(
                        (lambda n, uu: lambda: k_proj_unit(n, uu))(msb + 1, u))
                for nbp in (2 * msb + 2, 2 * msb + 3):
                    for half in range(2):
                        for u in range(4):
                            pending.append(
                                (lambda n, hh, uu: lambda:
                                 v_proj_unit(n, hh, uu))(nbp, half, u))
                for cb in range(4):
                    for qu in range(4):
                        pending.append(
                            (lambda m, c, u: lambda: q_proj_unit(m, c, u))(
                                msb + 1, cb, qu))

            n_steps = 4 * (4 * msb + 4)
            n_fill = len(pending)
            step = 0
            emitted = 0

            outT_t = outT_pool.tile([128, 2048], BF16, tag="outT")
            for p in range(4):
                # acc A/B rows 0:64 = half out, row 64 = denominator;
                # the half-b eviction shifts partitions 0:64 -> 64:128.
                accA = acc_psum.tile([128, 512], F32, tag="acc",
                                     name=f"accA{msb}{p}")
                accB = acc_psum.tile([128, 512], F32, tag="acc",
                                     name=f"accB{msb}{p}")
                njb = 4 * msb + 4
                for j in range(njb):
                    if j < 4 * msb:
                        moff, W = 0, 512
                    else:
                        t = j - 4 * msb
                        moff, W = 128 * t, 512 - 128 * t
                    first = (j == 0)
                    last = (j == njb - 1)
                    qlo = qT_sb[0:64, p * S + msb * 512 + moff:
                                p * S + msb * 512 + moff + W]
                    qhi = qT_sb[64:128, p * S + msb * 512 + moff:
                                p * S + msb * 512 + moff + W]
                    lt = lt_psum.tile([128, 1024], F32, tag="lt")
                    nc.tensor.matmul(
                        lt[:, 0:W],
                        kT_sb[0:64, j * 128:(j + 1) * 128], qlo,
                        start=True, stop=True, tile_position=(0, 0),
                    )
                    nc.tensor.matmul(
                        lt[:, 512:512 + W],
                        kT_sb[64:128, j * 128:(j + 1) * 128], qhi,
                        start=True, stop=True, tile_position=(64, 0),
                    )
                    pt = pt_pool.tile([128, 1024], BF16, tag="pt")
                    if W == 512:
                        nc.scalar.activation(pt[:], lt[:], Exp,
                                             scale=INV_SQRT_DIMK)
                    else:
                        # one strided activation covers both halves
                        nc.scalar.activation(
                            pt[:].rearrange("p (h w) -> p h w", h=2)[:, :, 0:W],
                            lt[:].rearrange("p (h w) -> p h w", h=2)[:, :, 0:W],
                            Exp, scale=INV_SQRT_DIMK)
                    if j >= 4 * msb:  # diagonal: mask the leading triangle
                        ptm = pt[:].rearrange("p (h w) -> p h w",
                                              h=2)[:, :, 0:128]
                        nc.vector.tensor_mul(
                            ptm, ptm,
                            mask_sb[:].rearrange("p (h w) -> p h w", h=2))
                    # attn @ v, 65-wide weights per half ([v | 1]): the
                    # denominator lands in psum row 64 of each acc bank.
                    nc.tensor.matmul(
                        accA[0:65, moff:moff + W],
                        v_sb[:, j * VW: j * VW + 65], pt[:, 0:W],
                        start=first, stop=last, tile_position=(0, 0),
                    )
                    nc.tensor.matmul(
                        accB[0:65, moff:moff + W],
                        v_sb[:, j * VW + 65: j * VW + VW], pt[:, 512:512 + W],
                        start=first, stop=last, tile_position=(0, 0),
                    )
                    step += 1
                    want = n_fill * step // n_steps
                    while pending and emitted < want:
                        pending.pop(0)()
                        emitted += 1
                # normalize: reciprocal rows -> PE outer-product broadcast
                # -> psum-eviction multiplies (with partition-window shift
                # for half b).
                recip_a = nrm_pool.tile([1, 512], BF16, tag="recipa",
                                        name=f"rca{msb}{p}")
                recip_b = nrm_pool.tile([1, 512], BF16, tag="recipb",
                                        name=f"rcb{msb}{p}")
                bcast = nrm_pool.tile([128, 512], F32, tag="bc",
                                      name=f"bc{msb}{p}")
                bcast_ps = pj_psum.tile([128, 512], F32, tag="ps",
                                        name=f"bcp{msb}{p}")
                with nc.allow_low_precision(reason="1/den bf16: 0.4% "
                                            "normalization err, tol 2e-2"):
                    nc.vector.reciprocal(recip_a[:], accA[64:65, :])
                    nc.vector.reciprocal(recip_b[:], accB[64:65, :])
                nc.tensor.matmul(
                    bcast_ps[0:64, :], ones_row[:], recip_a[:],
                    start=True, stop=True, tile_position=(0, 0),
                )
                nc.tensor.matmul(
                    bcast_ps[64:128, :], ones_row[:], recip_b[:],
                    start=True, stop=True, tile_position=(0, 64),
                )
                nc.vector.tensor_copy(bcast[:], bcast_ps[:])
                nc.vector.tensor_mul(
                    outT_t[0:64, p * 512:(p + 1) * 512],
                    accA[0:64, :], bcast[0:64, :],
                )
                nc.vector.tensor_mul(
                    outT_t[64:128, p * 512:(p + 1) * 512],
                    accB[0:64, :], bcast[64:128, :],
                )

            # Drain any unissued fillers, then queue this superblock's
            # output projection as fillers for the next one.
            for f in pending:
                f()
            pending = []
            o_t_box = {}
            for mb in range(4):
                for db in range(4):
                    pending.append(
                        (lambda m, ot, ob, a, b: lambda: wo_group(m, ot, ob, a, b))(
                            msb, outT_t, o_t_box, mb, db))
        for f in pending:
            f()


_NC_CACHE = {}


def get_nc():
    if "nc" not in _NC_CACHE:
        _NC_CACHE["nc"] = build_bass()
    return _NC_CACHE["nc"]


def kernel(inputs_q, inputs_k, inputs_v, Wq, bq, Wk, bk, Wv, bv, Wo, bo):
    inputs_q = np.asarray(inputs_q, np.float32)
    inputs_k = np.asarray(inputs_k, np.float32)
    inputs_v = np.asarray(inputs_v, np.float32)
    Wq = np.asarray(Wq, np.float32)
    Wk = np.asarray(Wk, np.float32)
    Wv = np.asarray(Wv, np.float32)
    Wo = np.asarray(Wo, np.float32)
    bq = np.asarray(bq, np.float32)
    bk = np.asarray(bk, np.float32)
    bv = np.asarray(bv, np.float32)
    bo = np.asarray(bo, np.float32)

    nc = get_nc()
    trimask = np.tile(np.triu(np.ones((128, 128), NPBF16)), (1, 2))

    xT = {}
    for b in range(B):
        xT[("q", b)] = np.ascontiguousarray(inputs_q[b].T.astype(NPBF16))
        xT[("k", b)] = np.ascontiguousarray(inputs_k[b].T.astype(NPBF16))
        xT[("v", b)] = np.ascontiguousarray(inputs_v[b].T.astype(NPBF16))

    in_maps = []
    for c in range(8):
        b = c // 4
        g0 = 2 * (c % 4)
        g1 = g0 + 1
        # pair-major channel permutation: (head p of g0, head p of g1), p=0..3
        perm = []
        for p in range(HPG):
            perm.extend(range(256 * g0 + 64 * p, 256 * g0 + 64 * p + 64))
            perm.extend(range(256 * g1 + 64 * p, 256 * g1 + 64 * p + 64))
        perm = np.array(perm)
        in_maps.append({
            "xqT": xT[("q", b)],
            "xkT": xT[("k", b)],
            "xvT": xT[("v", b)],
            "wq": np.ascontiguousarray(Wq[:, perm].astype(NPBF16)),
            "wk": np.ascontiguousarray(Wk[:, 64 * g0: 64 * g0 + 128].astype(NPBF16)),
            "wv": np.ascontiguousarray(Wv[:, 64 * g0: 64 * g0 + 128].astype(NPBF16)),
            "wo": np.ascontiguousarray(Wo[perm, :].astype(NPBF16)),
            "bq": np.ascontiguousarray(bq[perm].reshape(CPC, 1)),
            "bk": np.ascontiguousarray(bk[64 * g0: 64 * g0 + 128].reshape(128, 1)),
            "trimask": trimask,
        })

    res = run_bass_kernel_spmd(nc, in_maps, list(range(8)))

    # bv passes through (attention rows sum to 1): out += bv_expand @ Wo + bo
    bv_expand = np.repeat(bv.reshape(NKV, 1, HD), HPG, axis=1).reshape(D)
    corr = (bv_expand.astype(np.float64) @ Wo.astype(np.float64)) + bo

    outp = np.zeros((B, S, D), np.float64)
    for c in range(8):
        outp[c // 4] += res.results[c]["out"].astype(np.float64)
    outp += corr
    return outp.astype(np.float32)


# revision 6
# speedup vs baseline: 1.1017x; 1.1017x over previous
"""GroupedQueryAttention Trainium2 kernel (8-core SPMD), bf16.

Problem: B=2, S=2048, D=2048, 32 Q heads, 8 KV groups, head_dim=64.
  q = xq @ Wq + bq; k = xk @ Wk + bk; v = xv @ Wv + bv
  logits = q . k / sqrt(512), causal softmax, out = (attn @ v) @ Wo + bo

Sharding: one batch x two KV groups per core (2 batches x 4 group-pairs = 8).
Each core computes its 8 Q heads' attention and a partial output projection;
the host sums the 4 partials per batch and adds the bv/bo corrections (exact
because attention rows sum to 1).

Device-side design (sim ~0.30 ms/core vs 1.42 ms for the fp32 baseline):
- All matmul operands bf16 (1 PE cycle/row vs 4 for fp32); PSUM stays fp32.
  Host ships x/W inputs pre-transposed and bf16 (halves DMA bytes).
- Logits computed transposed (lT[n, m]) so attn@v needs no transpose and
  softmax denominators ride the attn@v matmuls: each half's v weights carry
  a 65th all-ones column ([v | 1]), landing sum_n p[n, m] in psum row 64.
  Normalization: DVE reciprocal -> PE outer-product broadcast -> fused
  psum-eviction multiply (half-b shifts partitions 0:64 -> 64:128).
- One Exp activation per n-block over a 2-bank [128, 1024] psum tile (both
  group halves, strided AP on diagonal tiles); causal masking by skipping
  n>m blocks, trimming diagonal widths, one strided triangle-mask multiply.
- Batched DMA (~45 transfers vs 327: HWDGE costs ~630 ns per instruction),
  first K/Q blocks split so the PE starts ~5 us in, out rows stored bf16.
- Software pipelining: projections stream just-in-time; K/V/Q blocks and
  the previous superblock's Wo groups are emitted as single-psum-bank
  "filler" units between attention j-steps, so the PE has backlog while
  the attnv chain waits on the scalar engine's exp.
- Bias adds on the DVE (tensor_scalar) to keep the scalar engine free for
  the exp stream; psum banks: pj(proj/Wo/bcast) 2 + logits 4 + acc 2 = 8.
"""

import math
import numpy as np
import ml_dtypes

import concourse.bass as bass
import concourse.mybir as mybir
from concourse import tile
from concourse.bass_utils import run_bass_kernel_spmd
from concourse.vector_clock import ScopedClock

F32 = mybir.dt.float32
BF16 = mybir.dt.bfloat16
NPBF16 = ml_dtypes.bfloat16
B, S, D = 2, 2048, 2048
NKV, HPG, HD = 8, 4, 64
DIMK = 512
CPC = 512                  # q channels per core (2 groups * 4 heads * 64)
KC = D // 128              # 16 k-chunks
MSB = S // 512             # 4 m-superblocks
NB = S // 128              # 16 n-blocks
VW = 130                   # v_sb cols per n-block: 64 va | 1 | 64 vb | 1
                           # (each half's weights end with an all-ones col,
                           # so both denominators ride the attn@v matmuls
                           # into psum row 64 of their acc bank)
INV_SQRT_DIMK = 1.0 / math.sqrt(float(DIMK))


# ---------------------------------------------------------------------------
# TileContext tail-drain patch: the bundled neuronxcc walrus rejects
# instructions carrying more than ~2 sync waits ("Too many sync wait
# commands"). Spread the kernel-tail waits over single-wait nops.
def _patched_drain_and_barrier(self, tick_clock, wait_clock):
    nc = self.nc
    collector = nc.sync.nop(nofuse=True)
    wait_clock.add_sem_waits(
        collector.ins, ScopedClock({None: tick_clock.global_clock})
    )
    si = collector.ins.sync_info
    waits = list(si.on_wait) if si is not None and si.on_wait else []
    if waits:
        collector.ins.sync_info = mybir.SyncInfo(
            on_wait=[waits[0]], on_update=list(si.on_update or [])
        )
        for w in waits[1:]:
            extra = nc.sync.nop(nofuse=True)
            extra.ins.sync_info = mybir.SyncInfo(on_wait=[w], on_update=[])
    nc.sync.drain()
    nc.all_engine_barrier()
    assert self.sems is not None
    popped = nc._tile_sem_poison_stack.pop()
    assert popped is self._sem_poison
    nc.clear_and_free_semaphores(list(self.sems.allocated().values()))
    nc.all_engine_barrier()


tile.TileContext._drain_and_barrier = _patched_drain_and_barrier


_MAXW = 1
_NOPID = [0]


def split_excess_waits(nc):
    """Walrus here encodes at most ~1-2 sync waits per instruction; move the
    excess onto preceding same-engine nops (engine order preserves timing)."""
    for f in nc.m.functions:
        for bb in f.blocks:
            out_list = []
            changed = False
            for inst in bb.instructions:
                si = getattr(inst, "sync_info", None)
                waits = list(si.on_wait) if si is not None and si.on_wait else []
                if len(waits) > _MAXW:
                    changed = True
                    for w in waits[:-_MAXW]:
                        _NOPID[0] += 1
                        nop = mybir.InstNoOp(
                            name=f"waitnop-{_NOPID[0]}", ins=[], outs=[],
                            engine=inst.engine,
                        )
                        nop.sync_info = mybir.SyncInfo(on_wait=[w], on_update=[])
                        out_list.append(nop)
                    inst.sync_info = mybir.SyncInfo(
                        on_wait=waits[-_MAXW:], on_update=list(si.on_update or [])
                    )
                out_list.append(inst)
            if changed:
                bb.instructions[:] = out_list
# ---------------------------------------------------------------------------


def build_bass():
    nc = bass.Bass()
    xqT = nc.dram_tensor("xqT", [D, S], BF16, kind="ExternalInput")
    xkT = nc.dram_tensor("xkT", [D, S], BF16, kind="ExternalInput")
    xvT = nc.dram_tensor("xvT", [D, S], BF16, kind="ExternalInput")
    wq = nc.dram_tensor("wq", [D, CPC], BF16, kind="ExternalInput")
    wk = nc.dram_tensor("wk", [D, 128], BF16, kind="ExternalInput")
    wv = nc.dram_tensor("wv", [D, 128], BF16, kind="ExternalInput")
    wo = nc.dram_tensor("wo", [CPC, D], BF16, kind="ExternalInput")
    bq = nc.dram_tensor("bq", [CPC, 1], F32, kind="ExternalInput")
    bk = nc.dram_tensor("bk", [128, 1], F32, kind="ExternalInput")
    trimask = nc.dram_tensor("trimask", [128, 256], BF16, kind="ExternalInput")
    out = nc.dram_tensor("out", [S, D], BF16, kind="ExternalOutput")

    from contextlib import ExitStack
    with tile.TileContext(nc) as tc, ExitStack() as ctx:
        build_body(ctx, tc, xqT, xkT, xvT, wq, wk, wv, wo, bq, bk, trimask, out)
    split_excess_waits(nc)
    return nc


def build_body(ctx, tc, xqT, xkT, xvT, wq, wk, wv, wo, bq, bk, trimask, out):
    nc = tc.nc
    Exp = mybir.ActivationFunctionType.Exp
    Ident = mybir.ActivationFunctionType.Identity

    const = ctx.enter_context(tc.tile_pool(name="const", bufs=1))
    wq_sb = const.tile([128, KC * CPC], BF16, tag="wq")        # [128, 8192]
    wk_sb = const.tile([128, KC * 128], BF16, tag="wk")        # [128, 2048]
    wv_sb = const.tile([128, KC * 128], BF16, tag="wv")        # [128, 2048]
    wo_sb = const.tile([128, 4 * D], BF16, tag="wo")           # [128, 8192]
    kT_sb = const.tile([128, S], BF16, tag="kT")               # [128, 2048]
    v_sb = const.tile([128, NB * VW], BF16, tag="v")           # [128, 2080]
    qT_sb = const.tile([128, 4 * S], BF16, tag="qT")           # [128, 8192]
    bq_sb = const.tile([128, 4], F32, tag="bq")
    bk_sb = const.tile([128, 1], F32, tag="bk")
    mask_sb = const.tile([128, 256], BF16, tag="mask")
    ones_row = const.tile([1, 64], BF16, tag="ones_row")

    # Weight / bias / mask loads — ordered by first use (wk gates the K
    # projection at t=0; wq/wo aren't needed until ~60/~90us in) so the
    # xk stream isn't stuck behind 4MB of late-use weights.
    nc.sync.dma_start(bk_sb[:], bk[:])
    nc.vector.memset(ones_row[:], 1.0)
    # all-ones columns interleaved in v_sb (denominators ride attn@v)
    for blk in range(NB):
        nc.vector.memset(v_sb[:, blk * VW + 64: blk * VW + 65], 1.0)
        nc.vector.memset(v_sb[:, blk * VW + 129: blk * VW + 130], 1.0)

    # psum budget: pj (K/V/Q proj + Wo + bcast) 2 + lt 4 + acc 2 = 8
    with tc.tile_pool(name="pj_psum", bufs=2, space="PSUM") as pj_psum, \
         tc.tile_pool(name="lt_psum", bufs=2, space="PSUM") as lt_psum, \
         tc.tile_pool(name="acc_psum", bufs=2, space="PSUM") as acc_psum, \
         tc.tile_pool(name="xin", bufs=2) as xin_pool, \
         tc.tile_pool(name="xvin", bufs=3) as xvin_pool, \
         tc.tile_pool(name="pt", bufs=4) as pt_pool, \
         tc.tile_pool(name="outT", bufs=2) as outT_pool, \
         tc.tile_pool(name="nrm", bufs=4) as nrm_pool, \
         tc.tile_pool(name="osb", bufs=2) as out_pool:

        xq_tiles = {}

        _qps_box = {}

        def q_proj_unit(msb, cb, u):
            if u == 0:
                _qps_box[(msb, cb)] = pj_psum.tile(
                    [128, 512], F32, tag="ps", name=f"psq{msb}{cb}")
            ps = _qps_box[(msb, cb)]
            xq_t = xq_tiles[msb]
            for kc in range(4 * u, 4 * u + 4):
                nc.tensor.matmul(
                    ps[:],
                    wq_sb[:, kc * CPC + cb * 128: kc * CPC + (cb + 1) * 128],
                    xq_t[:, kc * 512:(kc + 1) * 512],
                    start=(kc == 0), stop=(kc == KC - 1),
                )
            if u == 3:
                nc.vector.tensor_scalar_add(
                    qT_sb[:, cb * S + msb * 512: cb * S + (msb + 1) * 512],
                    ps[:], bq_sb[:, cb:cb + 1],
                )

        def q_proj_cb(msb, cb):
            for u in range(4):
                q_proj_unit(msb, cb, u)

        # --- single-bank filler units for projections -------------------
        # Each owner (a K block, a V half-pass, a Q cb-pass) accumulates in
        # one pj bank across its consecutive units, so units from different
        # owners can interleave with attention j-steps without deadlocking
        # the 2-bank pj ring.
        _kps_box = {}
        xk_tiles = {}

        def k_proj_unit(nsb, u):
            if u == 0:
                _kps_box[nsb] = pj_psum.tile([128, 512], F32, tag="ps",
                                             name=f"psk{nsb}")
            ps = _kps_box[nsb]
            xk_t = xk_tiles[nsb]
            for kc in range(4 * u, 4 * u + 4):
                nc.tensor.matmul(
                    ps[:], wk_sb[:, kc * 128:(kc + 1) * 128],
                    xk_t[:, kc * 512:(kc + 1) * 512],
                    start=(kc == 0), stop=(kc == KC - 1),
                )
            if u == 3:
                nc.vector.tensor_scalar_add(
                    kT_sb[:, nsb * 512:(nsb + 1) * 512], ps[:], bk_sb[:]
                )

        _vps_box = {}
        xv_tiles = {}

        def v_proj_unit(nbp, half, u):
            if u == 0:
                _vps_box[(nbp, half)] = pj_psum.tile(
                    [128, 128], F32, tag="ps", name=f"psv{nbp}{half}")
            ps = _vps_box[(nbp, half)]
            xv_t = xv_tiles[nbp]
            for kc in range(4 * u, 4 * u + 4):
                nc.tensor.matmul(
                    ps[:], xv_t[:, kc * 256 + half * 128:
                                kc * 256 + half * 128 + 128],
                    wv_sb[:, kc * 128:(kc + 1) * 128],
                    start=(kc == 0), stop=(kc == KC - 1),
                )
            if u == 3:
                blk = 2 * nbp + half
                nc.vector.tensor_copy(
                    v_sb[:, blk * VW: blk * VW + 64], ps[:, 0:64])
                nc.vector.tensor_copy(
                    v_sb[:, blk * VW + 65: blk * VW + 129], ps[:, 64:128])

        def xk_dma(nsb):
            xk_t = xin_pool.tile([128, KC * 512], BF16, tag="xk",
                                 name=f"xk{nsb}")
            nc.sync.dma_start(
                xk_t[:].rearrange("p (kc n) -> p kc n", kc=KC),
                xkT.rearrange("(kc p) n -> p kc n", p=128)[
                    :, :, nsb * 512:(nsb + 1) * 512],
            )
            xk_tiles[nsb] = xk_t

        def xv_dma(nbp):
            xv_t = xvin_pool.tile([128, KC * 256], BF16, tag="xv",
                                  name=f"xv{nbp}")
            nc.sync.dma_start(
                xv_t[:].rearrange("p (kc n) -> p kc n", kc=KC),
                xvT.rearrange("(kc p) n -> p kc n", p=128)[
                    :, :, nbp * 256:(nbp + 1) * 256],
            )
            xv_tiles[nbp] = xv_t

        # --- phase A: the minimum needed before attention msb0 ----------
        # K block 0 (4 sub-DMAs so the PE starts ~5us in), V blocks 0:4,
        # Q msb0 — everything else streams just-in-time as filler work.
        def wk_dma(h):
            nc.sync.dma_start(
                wk_sb[:, h * 4 * 128:(h + 1) * 4 * 128].rearrange(
                    "p (kc c) -> p kc c", kc=4),
                wk.rearrange("(kc p) c -> p kc c", p=128)[
                    :, h * 4:(h + 1) * 4],
            )

        with tc.tile_pool(name="xk0", bufs=4) as xk0_pool:
            xk0_t = [xk0_pool.tile([128, 4 * 512], BF16, tag="xk0",
                                   name=f"xk0{i}") for i in range(4)]
            for i in range(4):
                wk_dma(i)
                nc.sync.dma_start(
                    xk0_t[i][:].rearrange("p (kc n) -> p kc n", kc=4),
                    xkT.rearrange("(kc p) n -> p kc n", p=128)[
                        :, 4 * i: 4 * i + 4, 0:512],
                )
            nc.sync.dma_start(
                wv_sb[:].rearrange("p (kc c) -> p kc c", kc=KC),
                wv.rearrange("(kc p) c -> p kc c", p=128),
            )
            # xv0 ahead of the wq/xq0 halves: V0 is the first PE work
            # after K0; the Q chunks land while V0 runs.
            xv_dma(0)
            nc.sync.dma_start(
                bq_sb[:].rearrange("p (cb o) -> p cb o", cb=4),
                bq.rearrange("(cb p) o -> p cb o", p=128),
            )
            nc.sync.dma_start(mask_sb[:], trimask[:])
            xq_t0 = xin_pool.tile([128, KC * 512], BF16, tag="xk", name="xq0")
            xq_tiles[0] = xq_t0
            for h in range(2):
                nc.sync.dma_start(
                    xq_t0[:, h * 8 * 512:(h + 1) * 8 * 512].rearrange(
                        "p (kc n) -> p kc n", kc=8),
                    xqT.rearrange("(kc p) n -> p kc n", p=128)[
                        :, h * 8:(h + 1) * 8, 0:512],
                )
                nc.sync.dma_start(
                    wq_sb[:, h * 8 * CPC:(h + 1) * 8 * CPC].rearrange(
                        "p (kc c) -> p kc c", kc=8),
                    wq.rearrange("(kc p) c -> p kc c", p=128)[
                        :, h * 8:(h + 1) * 8],
                )
                if h == 0:
                    xv_dma(1)
            ps = pj_psum.tile([128, 512], F32, tag="ps", name="psk0")
            for kc in range(KC):
                nc.tensor.matmul(
                    ps[:], wk_sb[:, kc * 128:(kc + 1) * 128],
                    xk0_t[kc // 4][:, (kc % 4) * 512:(kc % 4 + 1) * 512],
                    start=(kc == 0), stop=(kc == KC - 1),
                )
            nc.vector.tensor_scalar_add(kT_sb[:, 0:512], ps[:], bk_sb[:])

        for half in range(2):
            for u in range(4):
                v_proj_unit(0, half, u)
        q_proj_cb(0, 0)
        for half in range(2):
            for u in range(4):
                v_proj_unit(1, half, u)
        q_proj_cb(0, 1)
        q_proj_cb(0, 2)
        q_proj_cb(0, 3)

        # Wo for one m-superblock as 16 filler units (one per mb/db pso
        # group); the out-row DMA rides the db==3 unit.
        def wo_group(msb, outT_t, o_t_box, mb, db):
            if db == 0:
                o_t_box[mb] = out_pool.tile([128, 2048], BF16, tag="osb",
                                            name=f"osb{msb}{mb}")
            o_t = o_t_box[mb]
            pso = pj_psum.tile([128, 512], F32, tag="ps",
                               name=f"pso{msb}{mb}{db}")
            for cb in range(4):
                nc.tensor.matmul(
                    pso[:],
                    outT_t[:, cb * 512 + mb * 128: cb * 512 + (mb + 1) * 128],
                    wo_sb[:, cb * D + db * 512: cb * D + (db + 1) * 512],
                    start=(cb == 0), stop=(cb == 3),
                )
            nc.vector.tensor_copy(o_t[:, db * 512:(db + 1) * 512], pso[:])
            if db == 3:
                nc.sync.dma_start(
                    out[msb * 512 + mb * 128: msb * 512 + (mb + 1) * 128, :],
                    o_t[:],
                )

        # Per m-superblock: attention with filler work (Wo groups of the
        # previous superblock + Q-projection passes of the next) emitted
        # between j-steps so the PE has backlog whenever the attnv chain
        # waits on the scalar engine's exp.
        pending = []   # filler closures, emitted oldest-first
        for msb in range(MSB):
            if msb + 1 < MSB:
                xk_dma(msb + 1)
                xv_dma(2 * msb + 2)
                xv_dma(2 * msb + 3)
                if msb == 0:
                    nc.sync.dma_start(
                        wo_sb[:].rearrange("p (cb d) -> p cb d", cb=4),
                        wo.rearrange("(cb p) d -> p cb d", p=128),
                    )
                xq_t = xin_pool.tile([128, KC * 512], BF16, tag="xk",
                                     name=f"xq{msb + 1}")
                nc.sync.dma_start(
                    xq_t[:].rearrange("p (kc n) -> p kc n", kc=KC),
                    xqT.rearrange("(kc p) n -> p kc n", p=128)[
                        :, :, (msb + 1) * 512:(msb + 2) * 512],
                )
                xq_tiles[msb + 1] = xq_t
                for u in range(4):
                    pending.append(
                        (lambda n, uu: lambda: k_proj_unit(n, uu))(msb + 1, u))
                for nbp in (2 * msb + 2, 2 * msb + 3):
                    for half in range(2):
                        for u in range(4):
                            pending.append(
                                (lambda n, hh, uu: lambda:
                                 v_proj_unit(n, hh, uu))(nbp, half, u))
                for cb in range(4):
                    for qu in range(4):
                        pending.append(
                            (lambda m, c, u: lambda: q_proj_unit(m, c, u))(
                                msb + 1, cb, qu))

            n_steps = 4 * (4 * msb + 4)
            n_fill = len(pending)
            step = 0
            emitted = 0

            outT_t = outT_pool.tile([128, 2048], BF16, tag="outT")
            for p in range(4):
                # acc A/B rows 0:64 = half out, row 64 = denominator;
                # the half-b eviction shifts partitions 0:64 -> 64:128.
                accA = acc_psum.tile([128, 512], F32, tag="acc",
                                     name=f"accA{msb}{p}")
                accB = acc_psum.tile([128, 512], F32, tag="acc",
                                     name=f"accB{msb}{p}")
                njb = 4 * msb + 4
                for j in range(njb):
                    if j < 4 * msb:
                        moff, W = 0, 512
                    else:
                        t = j - 4 * msb
                        moff, W = 128 * t, 512 - 128 * t
                    first = (j == 0)
                    last = (j == njb - 1)
                    qlo = qT_sb[0:64, p * S + msb * 512 + moff:
                                p * S + msb * 512 + moff + W]
                    qhi = qT_sb[64:128, p * S + msb * 512 + moff:
                                p * S + msb * 512 + moff + W]
                    lt = lt_psum.tile([128, 1024], F32, tag="lt")
                    nc.tensor.matmul(
                        lt[:, 0:W],
                        kT_sb[0:64, j * 128:(j + 1) * 128], qlo,
                        start=True, stop=True, tile_position=(0, 0),
                    )
                    nc.tensor.matmul(
                        lt[:, 512:512 + W],
                        kT_sb[64:128, j * 128:(j + 1) * 128], qhi,
                        start=True, stop=True, tile_position=(64, 0),
                    )
                    pt = pt_pool.tile([128, 1024], BF16, tag="pt")
                    if W == 512:
                        nc.scalar.activation(pt[:], lt[:], Exp,
                                             scale=INV_SQRT_DIMK)
                    else:
                        # one strided activation covers both halves
                        nc.scalar.activation(
                            pt[:].rearrange("p (h w) -> p h w", h=2)[:, :, 0:W],
                            lt[:].rearrange("p (h w) -> p h w", h=2)[:, :, 0:W],
                            Exp, scale=INV_SQRT_DIMK)
                    if j >= 4 * msb:  # diagonal: mask the leading triangle
                        ptm = pt[:].rearrange("p (h w) -> p h w",
                                              h=2)[:, :, 0:128]
                        nc.vector.tensor_mul(
                            ptm, ptm,
                            mask_sb[:].rearrange("p (h w) -> p h w", h=2))
                    # attn @ v, 65-wide weights per half ([v | 1]): the
                    # denominator lands in psum row 64 of each acc bank.
                    nc.tensor.matmul(
                        accA[0:65, moff:moff + W],
                        v_sb[:, j * VW: j * VW + 65], pt[:, 0:W],
                        start=first, stop=last, tile_position=(0, 0),
                    )
                    nc.tensor.matmul(
                        accB[0:65, moff:moff + W],
                        v_sb[:, j * VW + 65: j * VW + VW], pt[:, 512:512 + W],
                        start=first, stop=last, tile_position=(0, 0),
                    )
                    step += 1
                    want = n_fill * step // n_steps
                    while pending and emitted < want:
                        pending.pop(0)()
                        emitted += 1
                # normalize: reciprocal rows -> PE outer-product broadcast
                # -> psum-eviction multiplies (with partition-window shift
                # for half b).
                recip_a = nrm_pool.tile([1, 512], BF16, tag="recipa",
                                        name=f"rca{msb}{p}")
                recip_b = nrm_pool.tile([1, 512], BF16, tag="recipb",
                                        name=f"rcb{msb}{p}")
                bcast = nrm_pool.tile([128, 512], F32, tag="bc",
                                      name=f"bc{msb}{p}")
                bcast_ps = pj_psum.tile([128, 512], F32, tag="ps",
                                        name=f"bcp{msb}{p}")
                with nc.allow_low_precision(reason="1/den bf16: 0.4% "
                                            "normalization err, tol 2e-2"):
                    nc.vector.reciprocal(recip_a[:], accA[64:65, :])
                    nc.vector.reciprocal(recip_b[:], accB[64:65, :])
                nc.tensor.matmul(
                    bcast_ps[0:64, :], ones_row[:], recip_a[:],
                    start=True, stop=True, tile_position=(0, 0),
                )
                nc.tensor.matmul(
                    bcast_ps[64:128, :], ones_row[:], recip_b[:],
                    start=True, stop=True, tile_position=(0, 64),
                )
                nc.vector.tensor_copy(bcast[:], bcast_ps[:])
                nc.vector.tensor_mul(
                    outT_t[0:64, p * 512:(p + 1) * 512],
                    accA[0:64, :], bcast[0:64, :],
                )
                nc.vector.tensor_mul(
                    outT_t[64:128, p * 512:(p + 1) * 512],
                    accB[0:64, :], bcast[64:128, :],
                )

            # Drain any unissued fillers, then queue this superblock's
            # output projection as fillers for the next one.
            for f in pending:
                f()
            pending = []
            o_t_box = {}
            for mb in range(4):
                for db in range(4):
                    pending.append(
                        (lambda m, ot, ob, a, b: lambda: wo_group(m, ot, ob, a, b))(
                            msb, outT_t, o_t_box, mb, db))
        for f in pending:
            f()


_NC_CACHE = {}


def get_nc():
    if "nc" not in _NC_CACHE:
        _NC_CACHE["nc"] = build_bass()
    return _NC_CACHE["nc"]


def kernel(inputs_q, inputs_k, inputs_v, Wq, bq, Wk, bk, Wv, bv, Wo, bo):
    inputs_q = np.asarray(inputs_q, np.float32)
    inputs_k = np.asarray(inputs_k, np.float32)
    inputs_v = np.asarray(inputs_v, np.float32)
    Wq = np.asarray(Wq, np.float32)
    Wk = np.asarray(Wk, np.float32)
    Wv = np.asarray(Wv, np.float32)
    Wo = np.asarray(Wo, np.float32)
    bq = np.asarray(bq, np.float32)
    bk = np.asarray(bk, np.float32)
    bv = np.asarray(bv, np.float32)
    bo = np.asarray(bo, np.float32)

    nc = get_nc()
    trimask = np.tile(np.triu(np.ones((128, 128), NPBF16)), (1, 2))

    xT = {}
    for b in range(B):
        xT[("q", b)] = np.ascontiguousarray(inputs_q[b].T.astype(NPBF16))
        xT[("k", b)] = np.ascontiguousarray(inputs_k[b].T.astype(NPBF16))
        xT[("v", b)] = np.ascontiguousarray(inputs_v[b].T.astype(NPBF16))

    in_maps = []
    for c in range(8):
        b = c // 4
        g0 = 2 * (c % 4)
        g1 = g0 + 1
        # pair-major channel permutation: (head p of g0, head p of g1), p=0..3
        perm = []
        for p in range(HPG):
            perm.extend(range(256 * g0 + 64 * p, 256 * g0 + 64 * p + 64))
            perm.extend(range(256 * g1 + 64 * p, 256 * g1 + 64 * p + 64))
        perm = np.array(perm)
        in_maps.append({
            "xqT": xT[("q", b)],
            "xkT": xT[("k", b)],
            "xvT": xT[("v", b)],
            "wq": np.ascontiguousarray(Wq[:, perm].astype(NPBF16)),
            "wk": np.ascontiguousarray(Wk[:, 64 * g0: 64 * g0 + 128].astype(NPBF16)),
            "wv": np.ascontiguousarray(Wv[:, 64 * g0: 64 * g0 + 128].astype(NPBF16)),
            "wo": np.ascontiguousarray(Wo[perm, :].astype(NPBF16)),
            "bq": np.ascontiguousarray(bq[perm].reshape(CPC, 1)),
            "bk": np.ascontiguousarray(bk[64 * g0: 64 * g0 + 128].reshape(128, 1)),
            "trimask": trimask,
        })

    res = run_bass_kernel_spmd(nc, in_maps, list(range(8)))

    # bv passes through (attention rows sum to 1): out += bv_expand @ Wo + bo
    bv_expand = np.repeat(bv.reshape(NKV, 1, HD), HPG, axis=1).reshape(D)
    corr = (bv_expand.astype(np.float64) @ Wo.astype(np.float64)) + bo

    outp = np.zeros((B, S, D), np.float64)
    for c in range(8):
        outp[c // 4] += res.results[c]["out"].astype(np.float64)
    outp += corr
    return outp.astype(np.float32)


# revision 7
# speedup vs baseline: 1.1027x; 1.0009x over previous
"""GroupedQueryAttention Trainium2 kernel (8-core SPMD), bf16.

Problem: B=2, S=2048, D=2048, 32 Q heads, 8 KV groups, head_dim=64.
  q = xq @ Wq + bq; k = xk @ Wk + bk; v = xv @ Wv + bv
  logits = q . k / sqrt(512), causal softmax, out = (attn @ v) @ Wo + bo

Sharding: one batch x two KV groups per core (2 batches x 4 group-pairs = 8).
Each core computes its 8 Q heads' attention and a partial output projection;
the host sums the 4 partials per batch and adds the bv/bo corrections (exact
because attention rows sum to 1).

Device-side design (sim ~0.30 ms/core vs 1.42 ms for the fp32 baseline):
- All matmul operands bf16 (1 PE cycle/row vs 4 for fp32); PSUM stays fp32.
  Host ships x/W inputs pre-transposed and bf16 (halves DMA bytes).
- Logits computed transposed (lT[n, m]) so attn@v needs no transpose and
  softmax denominators ride the attn@v matmuls: each half's v weights carry
  a 65th all-ones column ([v | 1]), landing sum_n p[n, m] in psum row 64.
  Normalization: DVE reciprocal -> PE outer-product broadcast -> fused
  psum-eviction multiply (half-b shifts partitions 0:64 -> 64:128).
- One Exp activation per n-block over a 2-bank [128, 1024] psum tile (both
  group halves, strided AP on diagonal tiles); causal masking by skipping
  n>m blocks, trimming diagonal widths, one strided triangle-mask multiply.
- Batched DMA (~45 transfers vs 327: HWDGE costs ~630 ns per instruction),
  first K/Q blocks split so the PE starts ~5 us in, out rows stored bf16.
- Software pipelining: projections stream just-in-time; K/V/Q blocks and
  the previous superblock's Wo groups are emitted as single-psum-bank
  "filler" units between attention j-steps, so the PE has backlog while
  the attnv chain waits on the scalar engine's exp.
- Bias adds on the DVE (tensor_scalar) to keep the scalar engine free for
  the exp stream; psum banks: pj(proj/Wo/bcast) 2 + logits 4 + acc 2 = 8.
"""

import math
import numpy as np
import ml_dtypes

import concourse.bass as bass
import concourse.mybir as mybir
from concourse import tile
from concourse.bass_utils import run_bass_kernel_spmd
from concourse.vector_clock import ScopedClock

F32 = mybir.dt.float32
BF16 = mybir.dt.bfloat16
NPBF16 = ml_dtypes.bfloat16
B, S, D = 2, 2048, 2048
NKV, HPG, HD = 8, 4, 64
DIMK = 512
CPC = 512                  # q channels per core (2 groups * 4 heads * 64)
KC = D // 128              # 16 k-chunks
MSB = S // 512             # 4 m-superblocks
NB = S // 128              # 16 n-blocks
VW = 130                   # v_sb cols per n-block: 64 va | 1 | 64 vb | 1
                           # (each half's weights end with an all-ones col,
                           # so both denominators ride the attn@v matmuls
                           # into psum row 64 of their acc bank)
INV_SQRT_DIMK = 1.0 / math.sqrt(float(DIMK))


# ---------------------------------------------------------------------------
# TileContext tail-drain patch: the bundled neuronxcc walrus rejects
# instructions carrying more than ~2 sync waits ("Too many sync wait
# commands"). Spread the kernel-tail waits over single-wait nops.
def _patched_drain_and_barrier(self, tick_clock, wait_clock):
    nc = self.nc
    collector = nc.sync.nop(nofuse=True)
    wait_clock.add_sem_waits(
        collector.ins, ScopedClock({None: tick_clock.global_clock})
    )
    si = collector.ins.sync_info
    waits = list(si.on_wait) if si is not None and si.on_wait else []
    if waits:
        collector.ins.sync_info = mybir.SyncInfo(
            on_wait=[waits[0]], on_update=list(si.on_update or [])
        )
        for w in waits[1:]:
            extra = nc.sync.nop(nofuse=True)
            extra.ins.sync_info = mybir.SyncInfo(on_wait=[w], on_update=[])
    nc.sync.drain()
    nc.all_engine_barrier()
    assert self.sems is not None
    popped = nc._tile_sem_poison_stack.pop()
    assert popped is self._sem_poison
    nc.clear_and_free_semaphores(list(self.sems.allocated().values()))
    nc.all_engine_barrier()


tile.TileContext._drain_and_barrier = _patched_drain_and_barrier


_MAXW = 1
_NOPID = [0]


def split_excess_waits(nc):
    """Walrus here encodes at most ~1-2 sync waits per instruction; move the
    excess onto preceding same-engine nops (engine order preserves timing)."""
    for f in nc.m.functions:
        for bb in f.blocks:
            out_list = []
            changed = False
            for inst in bb.instructions:
                si = getattr(inst, "sync_info", None)
                waits = list(si.on_wait) if si is not None and si.on_wait else []
                if len(waits) > _MAXW:
                    changed = True
                    for w in waits[:-_MAXW]:
                        _NOPID[0] += 1
                        nop = mybir.InstNoOp(
                            name=f"waitnop-{_NOPID[0]}", ins=[], outs=[],
                            engine=inst.engine,
                        )
                        nop.sync_info = mybir.SyncInfo(on_wait=[w], on_update=[])
                        out_list.append(nop)
                    inst.sync_info = mybir.SyncInfo(
                        on_wait=waits[-_MAXW:], on_update=list(si.on_update or [])
                    )
                out_list.append(inst)
            if changed:
                bb.instructions[:] = out_list
# ---------------------------------------------------------------------------


def build_bass():
    nc = bass.Bass()
    xqT = nc.dram_tensor("xqT", [D, S], BF16, kind="ExternalInput")
    xkT = nc.dram_tensor("xkT", [D, S], BF16, kind="ExternalInput")
    xvT = nc.dram_tensor("xvT", [D, S], BF16, kind="ExternalInput")
    wq = nc.dram_tensor("wq", [D, CPC], BF16, kind="ExternalInput")
    # wk/wv ship pre-rearranged [128, kc*128] so the DMA is a straight
    # contiguous copy (the on-the-fly rearrange produced 256B runs, which
    # pay a 2x DMA latency penalty).
    wk = nc.dram_tensor("wk", [128, KC * 128], BF16, kind="ExternalInput")
    wv = nc.dram_tensor("wv", [128, KC * 128], BF16, kind="ExternalInput")
    wo = nc.dram_tensor("wo", [CPC, D], BF16, kind="ExternalInput")
    bq = nc.dram_tensor("bq", [CPC, 1], F32, kind="ExternalInput")
    bk = nc.dram_tensor("bk", [128, 1], F32, kind="ExternalInput")
    trimask = nc.dram_tensor("trimask", [128, 256], BF16, kind="ExternalInput")
    out = nc.dram_tensor("out", [S, D], BF16, kind="ExternalOutput")

    from contextlib import ExitStack
    with tile.TileContext(nc) as tc, ExitStack() as ctx:
        build_body(ctx, tc, xqT, xkT, xvT, wq, wk, wv, wo, bq, bk, trimask, out)
    split_excess_waits(nc)
    return nc


def build_body(ctx, tc, xqT, xkT, xvT, wq, wk, wv, wo, bq, bk, trimask, out):
    nc = tc.nc
    Exp = mybir.ActivationFunctionType.Exp
    Ident = mybir.ActivationFunctionType.Identity

    const = ctx.enter_context(tc.tile_pool(name="const", bufs=1))
    wq_sb = const.tile([128, KC * CPC], BF16, tag="wq")        # [128, 8192]
    wk_sb = const.tile([128, KC * 128], BF16, tag="wk")        # [128, 2048]
    wv_sb = const.tile([128, KC * 128], BF16, tag="wv")        # [128, 2048]
    wo_sb = const.tile([128, 4 * D], BF16, tag="wo")           # [128, 8192]
    kT_sb = const.tile([128, S], BF16, tag="kT")               # [128, 2048]
    v_sb = const.tile([128, NB * VW], BF16, tag="v")           # [128, 2080]
    qT_sb = const.tile([128, 4 * S], BF16, tag="qT")           # [128, 8192]
    bq_sb = const.tile([128, 4], F32, tag="bq")
    bk_sb = const.tile([128, 1], F32, tag="bk")
    mask_sb = const.tile([128, 256], BF16, tag="mask")
    ones_row = const.tile([1, 64], BF16, tag="ones_row")

    # Weight / bias / mask loads — ordered by first use (wk gates the K
    # projection at t=0; wq/wo aren't needed until ~60/~90us in) so the
    # xk stream isn't stuck behind 4MB of late-use weights.
    nc.sync.dma_start(bk_sb[:], bk[:])
    nc.vector.memset(ones_row[:], 1.0)
    # all-ones columns interleaved in v_sb (denominators ride attn@v)
    for blk in range(NB):
        nc.vector.memset(v_sb[:, blk * VW + 64: blk * VW + 65], 1.0)
        nc.vector.memset(v_sb[:, blk * VW + 129: blk * VW + 130], 1.0)

    # psum budget: pj (K/V/Q proj + Wo + bcast) 2 + lt 4 + acc 2 = 8
    with tc.tile_pool(name="pj_psum", bufs=2, space="PSUM") as pj_psum, \
         tc.tile_pool(name="lt_psum", bufs=2, space="PSUM") as lt_psum, \
         tc.tile_pool(name="acc_psum", bufs=2, space="PSUM") as acc_psum, \
         tc.tile_pool(name="xin", bufs=2) as xin_pool, \
         tc.tile_pool(name="xvin", bufs=3) as xvin_pool, \
         tc.tile_pool(name="pt", bufs=4) as pt_pool, \
         tc.tile_pool(name="outT", bufs=2) as outT_pool, \
         tc.tile_pool(name="nrm", bufs=4) as nrm_pool, \
         tc.tile_pool(name="osb", bufs=2) as out_pool:

        xq_tiles = {}

        _qps_box = {}

        def q_proj_unit(msb, cb, u):
            if u == 0:
                _qps_box[(msb, cb)] = pj_psum.tile(
                    [128, 512], F32, tag="ps", name=f"psq{msb}{cb}")
            ps = _qps_box[(msb, cb)]
            xq_t = xq_tiles[msb]
            for kc in range(4 * u, 4 * u + 4):
                nc.tensor.matmul(
                    ps[:],
                    wq_sb[:, kc * CPC + cb * 128: kc * CPC + (cb + 1) * 128],
                    xq_t[:, kc * 512:(kc + 1) * 512],
                    start=(kc == 0), stop=(kc == KC - 1),
                )
            if u == 3:
                nc.vector.tensor_scalar_add(
                    qT_sb[:, cb * S + msb * 512: cb * S + (msb + 1) * 512],
                    ps[:], bq_sb[:, cb:cb + 1],
                )

        def q_proj_cb(msb, cb):
            for u in range(4):
                q_proj_unit(msb, cb, u)

        # --- single-bank filler units for projections -------------------
        # Each owner (a K block, a V half-pass, a Q cb-pass) accumulates in
        # one pj bank across its consecutive units, so units from different
        # owners can interleave with attention j-steps without deadlocking
        # the 2-bank pj ring.
        _kps_box = {}
        xk_tiles = {}

        def k_proj_unit(nsb, u):
            if u == 0:
                _kps_box[nsb] = pj_psum.tile([128, 512], F32, tag="ps",
                                             name=f"psk{nsb}")
            ps = _kps_box[nsb]
            xk_t = xk_tiles[nsb]
            for kc in range(4 * u, 4 * u + 4):
                nc.tensor.matmul(
                    ps[:], wk_sb[:, kc * 128:(kc + 1) * 128],
                    xk_t[:, kc * 512:(kc + 1) * 512],
                    start=(kc == 0), stop=(kc == KC - 1),
                )
            if u == 3:
                nc.vector.tensor_scalar_add(
                    kT_sb[:, nsb * 512:(nsb + 1) * 512], ps[:], bk_sb[:]
                )

        _vps_box = {}
        xv_tiles = {}

        def v_proj_unit(nbp, half, u):
            if u == 0:
                _vps_box[(nbp, half)] = pj_psum.tile(
                    [128, 128], F32, tag="ps", name=f"psv{nbp}{half}")
            ps = _vps_box[(nbp, half)]
            xv_t = xv_tiles[nbp]
            for kc in range(4 * u, 4 * u + 4):
                nc.tensor.matmul(
                    ps[:], xv_t[:, kc * 256 + half * 128:
                                kc * 256 + half * 128 + 128],
                    wv_sb[:, kc * 128:(kc + 1) * 128],
                    start=(kc == 0), stop=(kc == KC - 1),
                )
            if u == 3:
                blk = 2 * nbp + half
                nc.vector.tensor_copy(
                    v_sb[:, blk * VW: blk * VW + 64], ps[:, 0:64])
                nc.vector.tensor_copy(
                    v_sb[:, blk * VW + 65: blk * VW + 129], ps[:, 64:128])

        def xk_dma(nsb):
            xk_t = xin_pool.tile([128, KC * 512], BF16, tag="xk",
                                 name=f"xk{nsb}")
            nc.sync.dma_start(
                xk_t[:].rearrange("p (kc n) -> p kc n", kc=KC),
                xkT.rearrange("(kc p) n -> p kc n", p=128)[
                    :, :, nsb * 512:(nsb + 1) * 512],
            )
            xk_tiles[nsb] = xk_t

        def xv_dma(nbp):
            xv_t = xvin_pool.tile([128, KC * 256], BF16, tag="xv",
                                  name=f"xv{nbp}")
            nc.sync.dma_start(
                xv_t[:].rearrange("p (kc n) -> p kc n", kc=KC),
                xvT.rearrange("(kc p) n -> p kc n", p=128)[
                    :, :, nbp * 256:(nbp + 1) * 256],
            )
            xv_tiles[nbp] = xv_t

        # --- phase A: the minimum needed before attention msb0 ----------
        # K block 0 (4 sub-DMAs so the PE starts ~5us in), V blocks 0:4,
        # Q msb0 — everything else streams just-in-time as filler work.
        def wk_dma(h):
            nc.sync.dma_start(
                wk_sb[:, h * 4 * 128:(h + 1) * 4 * 128],
                wk[:, h * 4 * 128:(h + 1) * 4 * 128],
            )

        with tc.tile_pool(name="xk0", bufs=4) as xk0_pool:
            xk0_t = [xk0_pool.tile([128, 4 * 512], BF16, tag="xk0",
                                   name=f"xk0{i}") for i in range(4)]
            for i in range(4):
                wk_dma(i)
                nc.sync.dma_start(
                    xk0_t[i][:].rearrange("p (kc n) -> p kc n", kc=4),
                    xkT.rearrange("(kc p) n -> p kc n", p=128)[
                        :, 4 * i: 4 * i + 4, 0:512],
                )
            nc.sync.dma_start(wv_sb[:], wv[:])
            # xv0 ahead of the wq/xq0 halves: V0 is the first PE work
            # after K0; the Q chunks land while V0 runs.
            xv_dma(0)
            nc.sync.dma_start(
                bq_sb[:].rearrange("p (cb o) -> p cb o", cb=4),
                bq.rearrange("(cb p) o -> p cb o", p=128),
            )
            nc.sync.dma_start(mask_sb[:], trimask[:])
            xq_t0 = xin_pool.tile([128, KC * 512], BF16, tag="xk", name="xq0")
            xq_tiles[0] = xq_t0
            for h in range(2):
                nc.sync.dma_start(
                    xq_t0[:, h * 8 * 512:(h + 1) * 8 * 512].rearrange(
                        "p (kc n) -> p kc n", kc=8),
                    xqT.rearrange("(kc p) n -> p kc n", p=128)[
                        :, h * 8:(h + 1) * 8, 0:512],
                )
                nc.sync.dma_start(
                    wq_sb[:, h * 8 * CPC:(h + 1) * 8 * CPC].rearrange(
                        "p (kc c) -> p kc c", kc=8),
                    wq.rearrange("(kc p) c -> p kc c", p=128)[
                        :, h * 8:(h + 1) * 8],
                )
                if h == 0:
                    xv_dma(1)
            ps = pj_psum.tile([128, 512], F32, tag="ps", name="psk0")
            for kc in range(KC):
                nc.tensor.matmul(
                    ps[:], wk_sb[:, kc * 128:(kc + 1) * 128],
                    xk0_t[kc // 4][:, (kc % 4) * 512:(kc % 4 + 1) * 512],
                    start=(kc == 0), stop=(kc == KC - 1),
                )
            nc.vector.tensor_scalar_add(kT_sb[:, 0:512], ps[:], bk_sb[:])

        for half in range(2):
            for u in range(4):
                v_proj_unit(0, half, u)
        q_proj_cb(0, 0)
        for half in range(2):
            for u in range(4):
                v_proj_unit(1, half, u)
        q_proj_cb(0, 1)
        q_proj_cb(0, 2)
        q_proj_cb(0, 3)

        # Wo for one m-superblock as 16 filler units (one per mb/db pso
        # group); the out-row DMA rides the db==3 unit.
        def wo_group(msb, outT_t, o_t_box, mb, db):
            if db == 0:
                o_t_box[mb] = out_pool.tile([128, 2048], BF16, tag="osb",
                                            name=f"osb{msb}{mb}")
            o_t = o_t_box[mb]
            pso = pj_psum.tile([128, 512], F32, tag="ps",
                               name=f"pso{msb}{mb}{db}")
            for cb in range(4):
                nc.tensor.matmul(
                    pso[:],
                    outT_t[:, cb * 512 + mb * 128: cb * 512 + (mb + 1) * 128],
                    wo_sb[:, cb * D + db * 512: cb * D + (db + 1) * 512],
                    start=(cb == 0), stop=(cb == 3),
                )
            nc.vector.tensor_copy(o_t[:, db * 512:(db + 1) * 512], pso[:])
            if db == 3:
                nc.sync.dma_start(
                    out[msb * 512 + mb * 128: msb * 512 + (mb + 1) * 128, :],
                    o_t[:],
                )

        # Per m-superblock: attention with filler work (Wo groups of the
        # previous superblock + Q-projection passes of the next) emitted
        # between j-steps so the PE has backlog whenever the attnv chain
        # waits on the scalar engine's exp.
        pending = []   # filler closures, emitted oldest-first
        for msb in range(MSB):
            if msb + 1 < MSB:
                xk_dma(msb + 1)
                xv_dma(2 * msb + 2)
                xv_dma(2 * msb + 3)
                if msb == 0:
                    nc.sync.dma_start(
                        wo_sb[:].rearrange("p (cb d) -> p cb d", cb=4),
                        wo.rearrange("(cb p) d -> p cb d", p=128),
                    )
                xq_t = xin_pool.tile([128, KC * 512], BF16, tag="xk",
                                     name=f"xq{msb + 1}")
                nc.sync.dma_start(
                    xq_t[:].rearrange("p (kc n) -> p kc n", kc=KC),
                    xqT.rearrange("(kc p) n -> p kc n", p=128)[
                        :, :, (msb + 1) * 512:(msb + 2) * 512],
                )
                xq_tiles[msb + 1] = xq_t
                for u in range(4):
                    pending.append(
                        (lambda n, uu: lambda: k_proj_unit(n, uu))(msb + 1, u))
                for nbp in (2 * msb + 2, 2 * msb + 3):
                    for half in range(2):
                        for u in range(4):
                            pending.append(
                                (lambda n, hh, uu: lambda:
                                 v_proj_unit(n, hh, uu))(nbp, half, u))
                for cb in range(4):
                    for qu in range(4):
                        pending.append(
                            (lambda m, c, u: lambda: q_proj_unit(m, c, u))(
                                msb + 1, cb, qu))

            n_steps = 4 * (4 * msb + 4)
            n_fill = len(pending)
            step = 0
            emitted = 0

            outT_t = outT_pool.tile([128, 2048], BF16, tag="outT")
            for p in range(4):
                # acc A/B rows 0:64 = half out, row 64 = denominator;
                # the half-b eviction shifts partitions 0:64 -> 64:128.
                accA = acc_psum.tile([128, 512], F32, tag="acc",
                                     name=f"accA{msb}{p}")
                accB = acc_psum.tile([128, 512], F32, tag="acc",
                                     name=f"accB{msb}{p}")
                njb = 4 * msb + 4
                for j in range(njb):
                    if j < 4 * msb:
                        moff, W = 0, 512
                    else:
                        t = j - 4 * msb
                        moff, W = 128 * t, 512 - 128 * t
                    first = (j == 0)
                    last = (j == njb - 1)
                    qlo = qT_sb[0:64, p * S + msb * 512 + moff:
                                p * S + msb * 512 + moff + W]
                    qhi = qT_sb[64:128, p * S + msb * 512 + moff:
                                p * S + msb * 512 + moff + W]
                    lt = lt_psum.tile([128, 1024], F32, tag="lt")
                    nc.tensor.matmul(
                        lt[:, 0:W],
                        kT_sb[0:64, j * 128:(j + 1) * 128], qlo,
                        start=True, stop=True, tile_position=(0, 0),
                    )
                    nc.tensor.matmul(
                        lt[:, 512:512 + W],
                        kT_sb[64:128, j * 128:(j + 1) * 128], qhi,
                        start=True, stop=True, tile_position=(64, 0),
                    )
                    pt = pt_pool.tile([128, 1024], BF16, tag="pt")
                    if W == 512:
                        nc.scalar.activation(pt[:], lt[:], Exp,
                                             scale=INV_SQRT_DIMK)
                    else:
                        # one strided activation covers both halves
                        nc.scalar.activation(
                            pt[:].rearrange("p (h w) -> p h w", h=2)[:, :, 0:W],
                            lt[:].rearrange("p (h w) -> p h w", h=2)[:, :, 0:W],
                            Exp, scale=INV_SQRT_DIMK)
                    if j >= 4 * msb:  # diagonal: mask the leading triangle
                        ptm = pt[:].rearrange("p (h w) -> p h w",
                                              h=2)[:, :, 0:128]
                        nc.vector.tensor_mul(
                            ptm, ptm,
                            mask_sb[:].rearrange("p (h w) -> p h w", h=2))
                    # attn @ v, 65-wide weights per half ([v | 1]): the
                    # denominator lands in psum row 64 of each acc bank.
                    nc.tensor.matmul(
                        accA[0:65, moff:moff + W],
                        v_sb[:, j * VW: j * VW + 65], pt[:, 0:W],
                        start=first, stop=last, tile_position=(0, 0),
                    )
                    nc.tensor.matmul(
                        accB[0:65, moff:moff + W],
                        v_sb[:, j * VW + 65: j * VW + VW], pt[:, 512:512 + W],
                        start=first, stop=last, tile_position=(0, 0),
                    )
                    step += 1
                    want = n_fill * step // n_steps
                    while pending and emitted < want:
                        pending.pop(0)()
                        emitted += 1
                # normalize: reciprocal rows -> PE outer-product broadcast
                # -> psum-eviction multiplies (with partition-window shift
                # for half b).
                recip_a = nrm_pool.tile([1, 512], BF16, tag="recipa",
                                        name=f"rca{msb}{p}")
                recip_b = nrm_pool.tile([1, 512], BF16, tag="recipb",
                                        name=f"rcb{msb}{p}")
                bcast = nrm_pool.tile([128, 512], F32, tag="bc",
                                      name=f"bc{msb}{p}")
                bcast_ps = pj_psum.tile([128, 512], F32, tag="ps",
                                        name=f"bcp{msb}{p}")
                with nc.allow_low_precision(reason="1/den bf16: 0.4% "
                                            "normalization err, tol 2e-2"):
                    nc.vector.reciprocal(recip_a[:], accA[64:65, :])
                    nc.vector.reciprocal(recip_b[:], accB[64:65, :])
                nc.tensor.matmul(
                    bcast_ps[0:64, :], ones_row[:], recip_a[:],
                    start=True, stop=True, tile_position=(0, 0),
                )
                nc.tensor.matmul(
                    bcast_ps[64:128, :], ones_row[:], recip_b[:],
                    start=True, stop=True, tile_position=(0, 64),
                )
                nc.vector.tensor_copy(bcast[:], bcast_ps[:])
                nc.vector.tensor_mul(
                    outT_t[0:64, p * 512:(p + 1) * 512],
                    accA[0:64, :], bcast[0:64, :],
                )
                nc.vector.tensor_mul(
                    outT_t[64:128, p * 512:(p + 1) * 512],
                    accB[0:64, :], bcast[64:128, :],
                )

            # Drain any unissued fillers, then queue this superblock's
            # output projection as fillers for the next one.
            for f in pending:
                f()
            pending = []
            o_t_box = {}
            for mb in range(4):
                for db in range(4):
                    pending.append(
                        (lambda m, ot, ob, a, b: lambda: wo_group(m, ot, ob, a, b))(
                            msb, outT_t, o_t_box, mb, db))
        for f in pending:
            f()


_NC_CACHE = {}


def get_nc():
    if "nc" not in _NC_CACHE:
        _NC_CACHE["nc"] = build_bass()
    return _NC_CACHE["nc"]


def kernel(inputs_q, inputs_k, inputs_v, Wq, bq, Wk, bk, Wv, bv, Wo, bo):
    inputs_q = np.asarray(inputs_q, np.float32)
    inputs_k = np.asarray(inputs_k, np.float32)
    inputs_v = np.asarray(inputs_v, np.float32)
    Wq = np.asarray(Wq, np.float32)
    Wk = np.asarray(Wk, np.float32)
    Wv = np.asarray(Wv, np.float32)
    Wo = np.asarray(Wo, np.float32)
    bq = np.asarray(bq, np.float32)
    bk = np.asarray(bk, np.float32)
    bv = np.asarray(bv, np.float32)
    bo = np.asarray(bo, np.float32)

    nc = get_nc()
    trimask = np.tile(np.triu(np.ones((128, 128), NPBF16)), (1, 2))

    xT = {}
    for b in range(B):
        xT[("q", b)] = np.ascontiguousarray(inputs_q[b].T.astype(NPBF16))
        xT[("k", b)] = np.ascontiguousarray(inputs_k[b].T.astype(NPBF16))
        xT[("v", b)] = np.ascontiguousarray(inputs_v[b].T.astype(NPBF16))

    in_maps = []
    for c in range(8):
        b = c // 4
        g0 = 2 * (c % 4)
        g1 = g0 + 1
        # pair-major channel permutation: (head p of g0, head p of g1), p=0..3
        perm = []
        for p in range(HPG):
            perm.extend(range(256 * g0 + 64 * p, 256 * g0 + 64 * p + 64))
            perm.extend(range(256 * g1 + 64 * p, 256 * g1 + 64 * p + 64))
        perm = np.array(perm)
        in_maps.append({
            "xqT": xT[("q", b)],
            "xkT": xT[("k", b)],
            "xvT": xT[("v", b)],
            "wq": np.ascontiguousarray(Wq[:, perm].astype(NPBF16)),
            "wk": np.ascontiguousarray(
                Wk[:, 64 * g0: 64 * g0 + 128].astype(NPBF16)
                .reshape(KC, 128, 128).transpose(1, 0, 2).reshape(128, -1)),
            "wv": np.ascontiguousarray(
                Wv[:, 64 * g0: 64 * g0 + 128].astype(NPBF16)
                .reshape(KC, 128, 128).transpose(1, 0, 2).reshape(128, -1)),
            "wo": np.ascontiguousarray(Wo[perm, :].astype(NPBF16)),
            "bq": np.ascontiguousarray(bq[perm].reshape(CPC, 1)),
            "bk": np.ascontiguousarray(bk[64 * g0: 64 * g0 + 128].reshape(128, 1)),
            "trimask": trimask,
        })

    res = run_bass_kernel_spmd(nc, in_maps, list(range(8)))

    # bv passes through (attention rows sum to 1): out += bv_expand @ Wo + bo
    bv_expand = np.repeat(bv.reshape(NKV, 1, HD), HPG, axis=1).reshape(D)
    corr = (bv_expand.astype(np.float64) @ Wo.astype(np.float64)) + bo

    outp = np.zeros((B, S, D), np.float64)
    for c in range(8):
        outp[c // 4] += res.results[c]["out"].astype(np.float64)
    outp += corr
    return outp.astype(np.float32)


# revision 8
# speedup vs baseline: 1.6375x; 1.4850x over previous
"""GroupedQueryAttention Trainium2 kernel (8-core SPMD), bf16.

Problem: B=2, S=2048, D=2048, 32 Q heads, 8 KV groups, head_dim=64.
  q = xq @ Wq + bq; k = xk @ Wk + bk; v = xv @ Wv + bv
  logits = q . k / sqrt(512), causal softmax, out = (attn @ v) @ Wo + bo

Sharding: one batch x two KV groups per core (2 batches x 4 group-pairs = 8).
Each core computes its 8 Q heads' attention and a partial output projection;
the host sums the 4 partials per batch and adds the bv/bo corrections (exact
because attention rows sum to 1).

Device-side design (sim ~0.30 ms/core vs 1.42 ms for the fp32 baseline):
- All matmul operands bf16 (1 PE cycle/row vs 4 for fp32); PSUM stays fp32.
  Host ships x/W inputs pre-transposed and bf16 (halves DMA bytes).
- Logits computed transposed (lT[n, m]) so attn@v needs no transpose and
  softmax denominators ride the attn@v matmuls: each half's v weights carry
  a 65th all-ones column ([v | 1]), landing sum_n p[n, m] in psum row 64.
  Normalization: DVE reciprocal -> PE outer-product broadcast -> fused
  psum-eviction multiply (half-b shifts partitions 0:64 -> 64:128).
- One Exp activation per n-block over a 2-bank [128, 1024] psum tile (both
  group halves, strided AP on diagonal tiles); causal masking by skipping
  n>m blocks, trimming diagonal widths, one strided triangle-mask multiply.
- Batched DMA (~45 transfers vs 327: HWDGE costs ~630 ns per instruction),
  first K/Q blocks split so the PE starts ~5 us in, out rows stored bf16.
- Software pipelining: projections stream just-in-time; K/V/Q blocks and
  the previous superblock's Wo groups are emitted as single-psum-bank
  "filler" units between attention j-steps, so the PE has backlog while
  the attnv chain waits on the scalar engine's exp.
- Bias adds on the DVE (tensor_scalar) to keep the scalar engine free for
  the exp stream; psum banks: pj(proj/Wo/bcast) 2 + logits 4 + acc 2 = 8.
"""

import math
import numpy as np
import ml_dtypes

import concourse.bass as bass
import concourse.mybir as mybir
from concourse import tile
from concourse.bass_utils import run_bass_kernel_spmd
from concourse.vector_clock import ScopedClock

F32 = mybir.dt.float32
BF16 = mybir.dt.bfloat16
NPBF16 = ml_dtypes.bfloat16
B, S, D = 2, 2048, 2048
NKV, HPG, HD = 8, 4, 64
DIMK = 512
CPC = 512                  # q channels per core (2 groups * 4 heads * 64)
KC = D // 128              # 16 k-chunks
MSB = S // 512             # 4 m-superblocks
NB = S // 128              # 16 n-blocks
VW = 130                   # v_sb cols per n-block: 64 va | 1 | 64 vb | 1
                           # (each half's weights end with an all-ones col,
                           # so both denominators ride the attn@v matmuls
                           # into psum row 64 of their acc bank)
INV_SQRT_DIMK = 1.0 / math.sqrt(float(DIMK))


# ---------------------------------------------------------------------------
# TileContext tail-drain patch: the bundled neuronxcc walrus rejects
# instructions carrying more than ~2 sync waits ("Too many sync wait
# commands"). Spread the kernel-tail waits over single-wait nops.
def _patched_drain_and_barrier(self, tick_clock, wait_clock):
    nc = self.nc
    collector = nc.sync.nop(nofuse=True)
    wait_clock.add_sem_waits(
        collector.ins, ScopedClock({None: tick_clock.global_clock})
    )
    si = collector.ins.sync_info
    waits = list(si.on_wait) if si is not None and si.on_wait else []
    if waits:
        collector.ins.sync_info = mybir.SyncInfo(
            on_wait=[waits[0]], on_update=list(si.on_update or [])
        )
        for w in waits[1:]:
            extra = nc.sync.nop(nofuse=True)
            extra.ins.sync_info = mybir.SyncInfo(on_wait=[w], on_update=[])
    nc.sync.drain()
    nc.all_engine_barrier()
    assert self.sems is not None
    popped = nc._tile_sem_poison_stack.pop()
    assert popped is self._sem_poison
    nc.clear_and_free_semaphores(list(self.sems.allocated().values()))
    nc.all_engine_barrier()


tile.TileContext._drain_and_barrier = _patched_drain_and_barrier


_MAXW = 1
_NOPID = [0]


def split_excess_waits(nc):
    """Walrus here encodes at most ~1-2 sync waits per instruction; move the
    excess onto preceding same-engine nops (engine order preserves timing)."""
    for f in nc.m.functions:
        for bb in f.blocks:
            out_list = []
            changed = False
            for inst in bb.instructions:
                si = getattr(inst, "sync_info", None)
                waits = list(si.on_wait) if si is not None and si.on_wait else []
                if len(waits) > _MAXW:
                    changed = True
                    for w in waits[:-_MAXW]:
                        _NOPID[0] += 1
                        nop = mybir.InstNoOp(
                            name=f"waitnop-{_NOPID[0]}", ins=[], outs=[],
                            engine=inst.engine,
                        )
                        nop.sync_info = mybir.SyncInfo(on_wait=[w], on_update=[])
                        out_list.append(nop)
                    inst.sync_info = mybir.SyncInfo(
                        on_wait=waits[-_MAXW:], on_update=list(si.on_update or [])
                    )
                out_list.append(inst)
            if changed:
                bb.instructions[:] = out_list
# ---------------------------------------------------------------------------


def build_bass():
    nc = bass.Bass()
    xqT = nc.dram_tensor("xqT", [D, S], BF16, kind="ExternalInput")
    xkT = nc.dram_tensor("xkT", [D, S], BF16, kind="ExternalInput")
    xvT = nc.dram_tensor("xvT", [D, S], BF16, kind="ExternalInput")
    wq = nc.dram_tensor("wq", [D, CPC], BF16, kind="ExternalInput")
    # wk/wv ship pre-rearranged [128, kc*128] so the DMA is a straight
    # contiguous copy (the on-the-fly rearrange produced 256B runs, which
    # pay a 2x DMA latency penalty).
    wk = nc.dram_tensor("wk", [128, KC * 128], BF16, kind="ExternalInput")
    wv = nc.dram_tensor("wv", [128, KC * 128], BF16, kind="ExternalInput")
    wo = nc.dram_tensor("wo", [CPC, D], BF16, kind="ExternalInput")
    bq = nc.dram_tensor("bq", [CPC, 1], F32, kind="ExternalInput")
    bk = nc.dram_tensor("bk", [128, 1], F32, kind="ExternalInput")
    trimask = nc.dram_tensor("trimask", [128, 256], BF16, kind="ExternalInput")
    out = nc.dram_tensor("out", [S, D], BF16, kind="ExternalOutput")

    from contextlib import ExitStack
    with tile.TileContext(nc) as tc, ExitStack() as ctx:
        build_body(ctx, tc, xqT, xkT, xvT, wq, wk, wv, wo, bq, bk, trimask, out)
    split_excess_waits(nc)
    return nc


def build_body(ctx, tc, xqT, xkT, xvT, wq, wk, wv, wo, bq, bk, trimask, out):
    nc = tc.nc
    Exp = mybir.ActivationFunctionType.Exp
    Ident = mybir.ActivationFunctionType.Identity

    const = ctx.enter_context(tc.tile_pool(name="const", bufs=1))
    wq_sb = const.tile([128, KC * CPC], BF16, tag="wq")        # [128, 8192]
    wk_sb = const.tile([128, KC * 128], BF16, tag="wk")        # [128, 2048]
    wv_sb = const.tile([128, KC * 128], BF16, tag="wv")        # [128, 2048]
    wo_sb = const.tile([128, 4 * D], BF16, tag="wo")           # [128, 8192]
    kT_sb = const.tile([128, S], BF16, tag="kT")               # [128, 2048]
    v_sb = const.tile([128, NB * VW], BF16, tag="v")           # [128, 2080]
    qT_sb = const.tile([128, 4 * S], BF16, tag="qT")           # [128, 8192]
    bq_sb = const.tile([128, 4], F32, tag="bq")
    bk_sb = const.tile([128, 1], F32, tag="bk")
    mask_sb = const.tile([128, 256], BF16, tag="mask")
    ones_row = const.tile([1, 64], BF16, tag="ones_row")

    # Weight / bias / mask loads — ordered by first use (wk gates the K
    # projection at t=0; wq/wo aren't needed until ~60/~90us in) so the
    # xk stream isn't stuck behind 4MB of late-use weights.
    nc.sync.dma_start(bk_sb[:], bk[:])
    nc.vector.memset(ones_row[:], 1.0)
    # all-ones columns interleaved in v_sb (denominators ride attn@v)
    for blk in range(NB):
        nc.vector.memset(v_sb[:, blk * VW + 64: blk * VW + 65], 1.0)
        nc.vector.memset(v_sb[:, blk * VW + 129: blk * VW + 130], 1.0)

    # psum budget: pj (K/V/Q proj + Wo + bcast) 2 + lt 4 + acc 2 = 8
    with tc.tile_pool(name="pj_psum", bufs=2, space="PSUM") as pj_psum, \
         tc.tile_pool(name="lt_psum", bufs=2, space="PSUM") as lt_psum, \
         tc.tile_pool(name="acc_psum", bufs=2, space="PSUM") as acc_psum, \
         tc.tile_pool(name="xin", bufs=2) as xin_pool, \
         tc.tile_pool(name="xvin", bufs=3) as xvin_pool, \
         tc.tile_pool(name="pt", bufs=4) as pt_pool, \
         tc.tile_pool(name="outT", bufs=2) as outT_pool, \
         tc.tile_pool(name="nrm", bufs=4) as nrm_pool, \
         tc.tile_pool(name="osb", bufs=2) as out_pool:

        xq_tiles = {}

        _qps_box = {}

        def q_proj_unit(msb, cb, u):
            if u == 0:
                _qps_box[(msb, cb)] = pj_psum.tile(
                    [128, 512], F32, tag="ps", name=f"psq{msb}{cb}")
            ps = _qps_box[(msb, cb)]
            xq_t = xq_tiles[msb]
            for kc in range(4 * u, 4 * u + 4):
                nc.tensor.matmul(
                    ps[:],
                    wq_sb[:, kc * CPC + cb * 128: kc * CPC + (cb + 1) * 128],
                    xq_t[:, kc * 512:(kc + 1) * 512],
                    start=(kc == 0), stop=(kc == KC - 1),
                )
            if u == 3:
                nc.vector.tensor_scalar_add(
                    qT_sb[:, cb * S + msb * 512: cb * S + (msb + 1) * 512],
                    ps[:], bq_sb[:, cb:cb + 1],
                )

        def q_proj_cb(msb, cb):
            for u in range(4):
                q_proj_unit(msb, cb, u)

        # --- single-bank filler units for projections -------------------
        # Each owner (a K block, a V half-pass, a Q cb-pass) accumulates in
        # one pj bank across its consecutive units, so units from different
        # owners can interleave with attention j-steps without deadlocking
        # the 2-bank pj ring.
        _kps_box = {}
        xk_tiles = {}

        def k_proj_unit(nsb, u):
            if u == 0:
                _kps_box[nsb] = pj_psum.tile([128, 512], F32, tag="ps",
                                             name=f"psk{nsb}")
            ps = _kps_box[nsb]
            xk_t = xk_tiles[nsb]
            for kc in range(4 * u, 4 * u + 4):
                nc.tensor.matmul(
                    ps[:], wk_sb[:, kc * 128:(kc + 1) * 128],
                    xk_t[:, kc * 512:(kc + 1) * 512],
                    start=(kc == 0), stop=(kc == KC - 1),
                )
            if u == 3:
                nc.vector.tensor_scalar_add(
                    kT_sb[:, nsb * 512:(nsb + 1) * 512], ps[:], bk_sb[:]
                )

        _vps_box = {}
        xv_tiles = {}

        def v_proj_unit(nbp, half, u):
            if u == 0:
                _vps_box[(nbp, half)] = pj_psum.tile(
                    [128, 128], F32, tag="ps", name=f"psv{nbp}{half}")
            ps = _vps_box[(nbp, half)]
            xv_t = xv_tiles[nbp]
            for kc in range(4 * u, 4 * u + 4):
                nc.tensor.matmul(
                    ps[:], xv_t[:, kc * 256 + half * 128:
                                kc * 256 + half * 128 + 128],
                    wv_sb[:, kc * 128:(kc + 1) * 128],
                    start=(kc == 0), stop=(kc == KC - 1),
                )
            if u == 3:
                blk = 2 * nbp + half
                nc.vector.tensor_copy(
                    v_sb[:, blk * VW: blk * VW + 64], ps[:, 0:64])
                nc.vector.tensor_copy(
                    v_sb[:, blk * VW + 65: blk * VW + 129], ps[:, 64:128])

        def xk_dma(nsb):
            xk_t = xin_pool.tile([128, KC * 512], BF16, tag="xk",
                                 name=f"xk{nsb}")
            nc.sync.dma_start(
                xk_t[:].rearrange("p (kc n) -> p kc n", kc=KC),
                xkT.rearrange("(kc p) n -> p kc n", p=128)[
                    :, :, nsb * 512:(nsb + 1) * 512],
            )
            xk_tiles[nsb] = xk_t

        def xv_dma(nbp):
            xv_t = xvin_pool.tile([128, KC * 256], BF16, tag="xv",
                                  name=f"xv{nbp}")
            nc.sync.dma_start(
                xv_t[:].rearrange("p (kc n) -> p kc n", kc=KC),
                xvT.rearrange("(kc p) n -> p kc n", p=128)[
                    :, :, nbp * 256:(nbp + 1) * 256],
            )
            xv_tiles[nbp] = xv_t

        # --- phase A: the minimum needed before attention msb0 ----------
        # K block 0 (4 sub-DMAs so the PE starts ~5us in), V blocks 0:4,
        # Q msb0 — everything else streams just-in-time as filler work.
        def wk_dma(h):
            nc.sync.dma_start(
                wk_sb[:, h * 4 * 128:(h + 1) * 4 * 128],
                wk[:, h * 4 * 128:(h + 1) * 4 * 128],
            )

        with tc.tile_pool(name="xk0", bufs=4) as xk0_pool:
            xk0_t = [xk0_pool.tile([128, 4 * 512], BF16, tag="xk0",
                                   name=f"xk0{i}") for i in range(4)]
            for i in range(4):
                wk_dma(i)
                nc.sync.dma_start(
                    xk0_t[i][:].rearrange("p (kc n) -> p kc n", kc=4),
                    xkT.rearrange("(kc p) n -> p kc n", p=128)[
                        :, 4 * i: 4 * i + 4, 0:512],
                )
            nc.sync.dma_start(wv_sb[:], wv[:])
            # xv0 ahead of the wq/xq0 halves: V0 is the first PE work
            # after K0; the Q chunks land while V0 runs.
            xv_dma(0)
            nc.sync.dma_start(
                bq_sb[:].rearrange("p (cb o) -> p cb o", cb=4),
                bq.rearrange("(cb p) o -> p cb o", p=128),
            )
            nc.sync.dma_start(mask_sb[:], trimask[:])
            xq_t0 = xin_pool.tile([128, KC * 512], BF16, tag="xk", name="xq0")
            xq_tiles[0] = xq_t0
            for h in range(2):
                nc.sync.dma_start(
                    xq_t0[:, h * 8 * 512:(h + 1) * 8 * 512].rearrange(
                        "p (kc n) -> p kc n", kc=8),
                    xqT.rearrange("(kc p) n -> p kc n", p=128)[
                        :, h * 8:(h + 1) * 8, 0:512],
                )
                nc.sync.dma_start(
                    wq_sb[:, h * 8 * CPC:(h + 1) * 8 * CPC].rearrange(
                        "p (kc c) -> p kc c", kc=8),
                    wq.rearrange("(kc p) c -> p kc c", p=128)[
                        :, h * 8:(h + 1) * 8],
                )
                if h == 0:
                    xv_dma(1)
            ps = pj_psum.tile([128, 512], F32, tag="ps", name="psk0")
            for kc in range(KC):
                nc.tensor.matmul(
                    ps[:], wk_sb[:, kc * 128:(kc + 1) * 128],
                    xk0_t[kc // 4][:, (kc % 4) * 512:(kc % 4 + 1) * 512],
                    start=(kc == 0), stop=(kc == KC - 1),
                )
            nc.vector.tensor_scalar_add(kT_sb[:, 0:512], ps[:], bk_sb[:])

        for half in range(2):
            for u in range(4):
                v_proj_unit(0, half, u)
        q_proj_cb(0, 0)
        for half in range(2):
            for u in range(4):
                v_proj_unit(1, half, u)
        q_proj_cb(0, 1)

        # Wo for one m-superblock as 16 filler units (one per mb/db pso
        # group); the out-row DMA rides the db==3 unit.
        def wo_group(msb, outT_t, o_t_box, mb, db):
            if db == 0:
                o_t_box[mb] = out_pool.tile([128, 2048], BF16, tag="osb",
                                            name=f"osb{msb}{mb}")
            o_t = o_t_box[mb]
            pso = pj_psum.tile([128, 512], F32, tag="ps",
                               name=f"pso{msb}{mb}{db}")
            for cb in range(4):
                nc.tensor.matmul(
                    pso[:],
                    outT_t[:, cb * 512 + mb * 128: cb * 512 + (mb + 1) * 128],
                    wo_sb[:, cb * D + db * 512: cb * D + (db + 1) * 512],
                    start=(cb == 0), stop=(cb == 3),
                )
            nc.vector.tensor_copy(o_t[:, db * 512:(db + 1) * 512], pso[:])
            if db == 3:
                nc.sync.dma_start(
                    out[msb * 512 + mb * 128: msb * 512 + (mb + 1) * 128, :],
                    o_t[:],
                )

        # Per m-superblock: attention with filler work (Wo groups of the
        # previous superblock + Q-projection passes of the next) emitted
        # between j-steps so the PE has backlog whenever the attnv chain
        # waits on the scalar engine's exp.
        pending = []   # filler closures, emitted oldest-first
        for msb in range(MSB):
            if msb == 0:
                # Q0 cb2/cb3 ride as fillers inside attn0 p0/p1 (attention
                # p reads only its own cb's qT slice, so p2/p3's data is
                # ready just in time and attn0 starts ~8us earlier).
                for cb in (2, 3):
                    for qu in range(4):
                        pending.append(
                            (lambda c, u: lambda: q_proj_unit(0, c, u))(
                                cb, qu))
            if msb + 1 < MSB:
                xk_dma(msb + 1)
                xv_dma(2 * msb + 2)
                xv_dma(2 * msb + 3)
                if msb == 0:
                    nc.sync.dma_start(
                        wo_sb[:].rearrange("p (cb d) -> p cb d", cb=4),
                        wo.rearrange("(cb p) d -> p cb d", p=128),
                    )
                xq_t = xin_pool.tile([128, KC * 512], BF16, tag="xk",
                                     name=f"xq{msb + 1}")
                nc.sync.dma_start(
                    xq_t[:].rearrange("p (kc n) -> p kc n", kc=KC),
                    xqT.rearrange("(kc p) n -> p kc n", p=128)[
                        :, :, (msb + 1) * 512:(msb + 2) * 512],
                )
                xq_tiles[msb + 1] = xq_t
                for u in range(4):
                    pending.append(
                        (lambda n, uu: lambda: k_proj_unit(n, uu))(msb + 1, u))
                for nbp in (2 * msb + 2, 2 * msb + 3):
                    for half in range(2):
                        for u in range(4):
                            pending.append(
                                (lambda n, hh, uu: lambda:
                                 v_proj_unit(n, hh, uu))(nbp, half, u))
                for cb in range(4):
                    for qu in range(4):
                        pending.append(
                            (lambda m, c, u: lambda: q_proj_unit(m, c, u))(
                                msb + 1, cb, qu))

            n_steps = 4 * (4 * msb + 4)
            n_fill = len(pending)
            step = 0
            emitted = 0

            outT_t = outT_pool.tile([128, 2048], BF16, tag="outT")
            for p in range(4):
                # acc A/B rows 0:64 = half out, row 64 = denominator;
                # the half-b eviction shifts partitions 0:64 -> 64:128.
                accA = acc_psum.tile([128, 512], F32, tag="acc",
                                     name=f"accA{msb}{p}")
                accB = acc_psum.tile([128, 512], F32, tag="acc",
                                     name=f"accB{msb}{p}")
                njb = 4 * msb + 4
                for j in range(njb):
                    if j < 4 * msb:
                        moff, W = 0, 512
                    else:
                        t = j - 4 * msb
                        moff, W = 128 * t, 512 - 128 * t
                    first = (j == 0)
                    last = (j == njb - 1)
                    qlo = qT_sb[0:64, p * S + msb * 512 + moff:
                                p * S + msb * 512 + moff + W]
                    qhi = qT_sb[64:128, p * S + msb * 512 + moff:
                                p * S + msb * 512 + moff + W]
                    lt = lt_psum.tile([128, 1024], F32, tag="lt")
                    nc.tensor.matmul(
                        lt[:, 0:W],
                        kT_sb[0:64, j * 128:(j + 1) * 128], qlo,
                        start=True, stop=True, tile_position=(0, 0),
                    )
                    nc.tensor.matmul(
                        lt[:, 512:512 + W],
                        kT_sb[64:128, j * 128:(j + 1) * 128], qhi,
                        start=True, stop=True, tile_position=(64, 0),
                    )
                    pt = pt_pool.tile([128, 1024], BF16, tag="pt")
                    if W == 512:
                        nc.scalar.activation(pt[:], lt[:], Exp,
                                             scale=INV_SQRT_DIMK)
                    else:
                        # one strided activation covers both halves
                        nc.scalar.activation(
                            pt[:].rearrange("p (h w) -> p h w", h=2)[:, :, 0:W],
                            lt[:].rearrange("p (h w) -> p h w", h=2)[:, :, 0:W],
                            Exp, scale=INV_SQRT_DIMK)
                    if j >= 4 * msb:  # diagonal: mask the leading triangle
                        ptm = pt[:].rearrange("p (h w) -> p h w",
                                              h=2)[:, :, 0:128]
                        nc.vector.tensor_mul(
                            ptm, ptm,
                            mask_sb[:].rearrange("p (h w) -> p h w", h=2))
                    # attn @ v, 65-wide weights per half ([v | 1]): the
                    # denominator lands in psum row 64 of each acc bank.
                    nc.tensor.matmul(
                        accA[0:65, moff:moff + W],
                        v_sb[:, j * VW: j * VW + 65], pt[:, 0:W],
                        start=first, stop=last, tile_position=(0, 0),
                    )
                    nc.tensor.matmul(
                        accB[0:65, moff:moff + W],
                        v_sb[:, j * VW + 65: j * VW + VW], pt[:, 512:512 + W],
                        start=first, stop=last, tile_position=(0, 0),
                    )
                    step += 1
                    want = n_fill * step // n_steps
                    while pending and emitted < want:
                        pending.pop(0)()
                        emitted += 1
                # normalize: reciprocal rows -> PE outer-product broadcast
                # -> psum-eviction multiplies (with partition-window shift
                # for half b).
                recip_a = nrm_pool.tile([1, 512], BF16, tag="recipa",
                                        name=f"rca{msb}{p}")
                recip_b = nrm_pool.tile([1, 512], BF16, tag="recipb",
                                        name=f"rcb{msb}{p}")
                bcast = nrm_pool.tile([128, 512], F32, tag="bc",
                                      name=f"bc{msb}{p}")
                bcast_ps = pj_psum.tile([128, 512], F32, tag="ps",
                                        name=f"bcp{msb}{p}")
                with nc.allow_low_precision(reason="1/den bf16: 0.4% "
                                            "normalization err, tol 2e-2"):
                    nc.vector.reciprocal(recip_a[:], accA[64:65, :])
                    nc.vector.reciprocal(recip_b[:], accB[64:65, :])
                nc.tensor.matmul(
                    bcast_ps[0:64, :], ones_row[:], recip_a[:],
                    start=True, stop=True, tile_position=(0, 0),
                )
                nc.tensor.matmul(
                    bcast_ps[64:128, :], ones_row[:], recip_b[:],
                    start=True, stop=True, tile_position=(0, 64),
                )
                nc.vector.tensor_copy(bcast[:], bcast_ps[:])
                nc.vector.tensor_mul(
                    outT_t[0:64, p * 512:(p + 1) * 512],
                    accA[0:64, :], bcast[0:64, :],
                )
                nc.vector.tensor_mul(
                    outT_t[64:128, p * 512:(p + 1) * 512],
                    accB[0:64, :], bcast[64:128, :],
                )

            # Drain any unissued fillers, then queue this superblock's
            # output projection as fillers for the next one.
            for f in pending:
                f()
            pending = []
            o_t_box = {}
            for mb in range(4):
                for db in range(4):
                    pending.append(
                        (lambda m, ot, ob, a, b: lambda: wo_group(m, ot, ob, a, b))(
                            msb, outT_t, o_t_box, mb, db))
        for f in pending:
            f()


_NC_CACHE = {}


def get_nc():
    if "nc" not in _NC_CACHE:
        _NC_CACHE["nc"] = build_bass()
    return _NC_CACHE["nc"]


def kernel(inputs_q, inputs_k, inputs_v, Wq, bq, Wk, bk, Wv, bv, Wo, bo):
    inputs_q = np.asarray(inputs_q, np.float32)
    inputs_k = np.asarray(inputs_k, np.float32)
    inputs_v = np.asarray(inputs_v, np.float32)
    Wq = np.asarray(Wq, np.float32)
    Wk = np.asarray(Wk, np.float32)
    Wv = np.asarray(Wv, np.float32)
    Wo = np.asarray(Wo, np.float32)
    bq = np.asarray(bq, np.float32)
    bk = np.asarray(bk, np.float32)
    bv = np.asarray(bv, np.float32)
    bo = np.asarray(bo, np.float32)

    nc = get_nc()
    trimask = np.tile(np.triu(np.ones((128, 128), NPBF16)), (1, 2))

    xT = {}
    for b in range(B):
        xT[("q", b)] = np.ascontiguousarray(inputs_q[b].T.astype(NPBF16))
        xT[("k", b)] = np.ascontiguousarray(inputs_k[b].T.astype(NPBF16))
        xT[("v", b)] = np.ascontiguousarray(inputs_v[b].T.astype(NPBF16))

    in_maps = []
    for c in range(8):
        b = c // 4
        g0 = 2 * (c % 4)
        g1 = g0 + 1
        # pair-major channel permutation: (head p of g0, head p of g1), p=0..3
        perm = []
        for p in range(HPG):
            perm.extend(range(256 * g0 + 64 * p, 256 * g0 + 64 * p + 64))
            perm.extend(range(256 * g1 + 64 * p, 256 * g1 + 64 * p + 64))
        perm = np.array(perm)
        in_maps.append({
            "xqT": xT[("q", b)],
            "xkT": xT[("k", b)],
            "xvT": xT[("v", b)],
            "wq": np.ascontiguousarray(Wq[:, perm].astype(NPBF16)),
            "wk": np.ascontiguousarray(
                Wk[:, 64 * g0: 64 * g0 + 128].astype(NPBF16)
                .reshape(KC, 128, 128).transpose(1, 0, 2).reshape(128, -1)),
            "wv": np.ascontiguousarray(
                Wv[:, 64 * g0: 64 * g0 + 128].astype(NPBF16)
                .reshape(KC, 128, 128).transpose(1, 0, 2).reshape(128, -1)),
            "wo": np.ascontiguousarray(Wo[perm, :].astype(NPBF16)),
            "bq": np.ascontiguousarray(bq[perm].reshape(CPC, 1)),
            "bk": np.ascontiguousarray(bk[64 * g0: 64 * g0 + 128].reshape(128, 1)),
            "trimask": trimask,
        })

    res = run_bass_kernel_spmd(nc, in_maps, list(range(8)))

    # bv passes through (attention rows sum to 1): out += bv_expand @ Wo + bo
    bv_expand = np.repeat(bv.reshape(NKV, 1, HD), HPG, axis=1).reshape(D)
    corr = (bv_expand.astype(np.float64) @ Wo.astype(np.float64)) + bo

    outp = np.zeros((B, S, D), np.float64)
    for c in range(8):
        outp[c // 4] += res.results[c]["out"].astype(np.float64)
    outp += corr
    return outp.astype(np.float32)


# revision 9
# speedup vs baseline: 1.7642x; 1.0774x over previous
"""GroupedQueryAttention Trainium2 kernel (8-core SPMD), bf16.

Problem: B=2, S=2048, D=2048, 32 Q heads, 8 KV groups, head_dim=64.
  q = xq @ Wq + bq; k = xk @ Wk + bk; v = xv @ Wv + bv
  logits = q . k / sqrt(512), causal softmax, out = (attn @ v) @ Wo + bo

Sharding: one batch x two KV groups per core (2 batches x 4 group-pairs = 8).
Each core computes its 8 Q heads' attention and a partial output projection;
the host sums the 4 partials per batch and adds the bv/bo corrections (exact
because attention rows sum to 1).

Device-side design (sim ~0.30 ms/core vs 1.42 ms for the fp32 baseline):
- All matmul operands bf16 (1 PE cycle/row vs 4 for fp32); PSUM stays fp32.
  Host ships x/W inputs pre-transposed and bf16 (halves DMA bytes).
- Logits computed transposed (lT[n, m]) so attn@v needs no transpose and
  softmax denominators ride the attn@v matmuls: each half's v weights carry
  a 65th all-ones column ([v | 1]), landing sum_n p[n, m] in psum row 64.
  Normalization: DVE reciprocal -> PE outer-product broadcast -> fused
  psum-eviction multiply (half-b shifts partitions 0:64 -> 64:128).
- One Exp activation per n-block over a 2-bank [128, 1024] psum tile (both
  group halves, strided AP on diagonal tiles); causal masking by skipping
  n>m blocks, trimming diagonal widths, one strided triangle-mask multiply.
- Batched DMA (~45 transfers vs 327: HWDGE costs ~630 ns per instruction),
  first K/Q blocks split so the PE starts ~5 us in, out rows stored bf16.
- Software pipelining: projections stream just-in-time; K/V/Q blocks and
  the previous superblock's Wo groups are emitted as single-psum-bank
  "filler" units between attention j-steps, so the PE has backlog while
  the attnv chain waits on the scalar engine's exp.
- Bias adds on the DVE (tensor_scalar) to keep the scalar engine free for
  the exp stream; psum banks: pj(proj/Wo/bcast) 2 + logits 4 + acc 2 = 8.
"""

import math
import numpy as np
import ml_dtypes

import concourse.bass as bass
import concourse.mybir as mybir
from concourse import tile
from concourse.bass_utils import run_bass_kernel_spmd
from concourse.vector_clock import ScopedClock

F32 = mybir.dt.float32
BF16 = mybir.dt.bfloat16
NPBF16 = ml_dtypes.bfloat16
B, S, D = 2, 2048, 2048
NKV, HPG, HD = 8, 4, 64
DIMK = 512
CPC = 512                  # q channels per core (2 groups * 4 heads * 64)
KC = D // 128              # 16 k-chunks
MSB = S // 512             # 4 m-superblocks
NB = S // 128              # 16 n-blocks
VW = 130                   # v_sb cols per n-block: 64 va | 1 | 64 vb | 1
                           # (each half's weights end with an all-ones col,
                           # so both denominators ride the attn@v matmuls
                           # into psum row 64 of their acc bank)
INV_SQRT_DIMK = 1.0 / math.sqrt(float(DIMK))


# ---------------------------------------------------------------------------
# TileContext tail-drain patch: the bundled neuronxcc walrus rejects
# instructions carrying more than ~2 sync waits ("Too many sync wait
# commands"). Spread the kernel-tail waits over single-wait nops.
def _patched_drain_and_barrier(self, tick_clock, wait_clock):
    nc = self.nc
    collector = nc.sync.nop(nofuse=True)
    wait_clock.add_sem_waits(
        collector.ins, ScopedClock({None: tick_clock.global_clock})
    )
    si = collector.ins.sync_info
    waits = list(si.on_wait) if si is not None and si.on_wait else []
    if waits:
        collector.ins.sync_info = mybir.SyncInfo(
            on_wait=[waits[0]], on_update=list(si.on_update or [])
        )
        for w in waits[1:]:
            extra = nc.sync.nop(nofuse=True)
            extra.ins.sync_info = mybir.SyncInfo(on_wait=[w], on_update=[])
    nc.sync.drain()
    nc.all_engine_barrier()
    assert self.sems is not None
    popped = nc._tile_sem_poison_stack.pop()
    assert popped is self._sem_poison
    nc.clear_and_free_semaphores(list(self.sems.allocated().values()))
    nc.all_engine_barrier()


tile.TileContext._drain_and_barrier = _patched_drain_and_barrier


_MAXW = 1
_NOPID = [0]


def split_excess_waits(nc):
    """Walrus here encodes at most ~1-2 sync waits per instruction; move the
    excess onto preceding same-engine nops (engine order preserves timing)."""
    for f in nc.m.functions:
        for bb in f.blocks:
            out_list = []
            changed = False
            for inst in bb.instructions:
                si = getattr(inst, "sync_info", None)
                waits = list(si.on_wait) if si is not None and si.on_wait else []
                if len(waits) > _MAXW:
                    changed = True
                    for w in waits[:-_MAXW]:
                        _NOPID[0] += 1
                        nop = mybir.InstNoOp(
                            name=f"waitnop-{_NOPID[0]}", ins=[], outs=[],
                            engine=inst.engine,
                        )
                        nop.sync_info = mybir.SyncInfo(on_wait=[w], on_update=[])
                        out_list.append(nop)
                    inst.sync_info = mybir.SyncInfo(
                        on_wait=waits[-_MAXW:], on_update=list(si.on_update or [])
                    )
                out_list.append(inst)
            if changed:
                bb.instructions[:] = out_list
# ---------------------------------------------------------------------------


def build_bass():
    nc = bass.Bass()
    xqT = nc.dram_tensor("xqT", [D, S], BF16, kind="ExternalInput")
    xkT = nc.dram_tensor("xkT", [D, S], BF16, kind="ExternalInput")
    xvT = nc.dram_tensor("xvT", [D, S], BF16, kind="ExternalInput")
    wq = nc.dram_tensor("wq", [D, CPC], BF16, kind="ExternalInput")
    # wk/wv ship pre-rearranged [128, kc*128] so the DMA is a straight
    # contiguous copy (the on-the-fly rearrange produced 256B runs, which
    # pay a 2x DMA latency penalty).
    wk = nc.dram_tensor("wk", [128, KC * 128], BF16, kind="ExternalInput")
    wv = nc.dram_tensor("wv", [128, KC * 128], BF16, kind="ExternalInput")
    wo = nc.dram_tensor("wo", [CPC, D], BF16, kind="ExternalInput")
    bq = nc.dram_tensor("bq", [CPC, 1], F32, kind="ExternalInput")
    bk = nc.dram_tensor("bk", [128, 1], F32, kind="ExternalInput")
    trimask = nc.dram_tensor("trimask", [128, 256], BF16, kind="ExternalInput")
    out = nc.dram_tensor("out", [S, D], BF16, kind="ExternalOutput")

    from contextlib import ExitStack
    with tile.TileContext(nc) as tc, ExitStack() as ctx:
        build_body(ctx, tc, xqT, xkT, xvT, wq, wk, wv, wo, bq, bk, trimask, out)
    split_excess_waits(nc)
    return nc


def build_body(ctx, tc, xqT, xkT, xvT, wq, wk, wv, wo, bq, bk, trimask, out):
    nc = tc.nc
    Exp = mybir.ActivationFunctionType.Exp
    Ident = mybir.ActivationFunctionType.Identity

    const = ctx.enter_context(tc.tile_pool(name="const", bufs=1))
    wq_sb = const.tile([128, KC * CPC], BF16, tag="wq")        # [128, 8192]
    wk_sb = const.tile([128, KC * 128], BF16, tag="wk")        # [128, 2048]
    wv_sb = const.tile([128, KC * 128], BF16, tag="wv")        # [128, 2048]
    wo_sb = const.tile([128, 4 * D], BF16, tag="wo")           # [128, 8192]
    kT_sb = const.tile([128, S], BF16, tag="kT")               # [128, 2048]
    v_sb = const.tile([128, NB * VW], BF16, tag="v")           # [128, 2080]
    qT_sb = const.tile([128, 4 * S], BF16, tag="qT")           # [128, 8192]
    bq_sb = const.tile([128, 4], F32, tag="bq")
    bk_sb = const.tile([128, 1], F32, tag="bk")
    mask_sb = const.tile([128, 256], BF16, tag="mask")
    ones_row = const.tile([1, 64], BF16, tag="ones_row")

    # Weight / bias / mask loads — ordered by first use (wk gates the K
    # projection at t=0; wq/wo aren't needed until ~60/~90us in) so the
    # xk stream isn't stuck behind 4MB of late-use weights.
    nc.vector.memset(ones_row[:], 1.0)
    # all-ones columns interleaved in v_sb (denominators ride attn@v)
    for blk in range(NB):
        nc.vector.memset(v_sb[:, blk * VW + 64: blk * VW + 65], 1.0)
        nc.vector.memset(v_sb[:, blk * VW + 129: blk * VW + 130], 1.0)

    # psum budget: pj (K/V/Q proj + Wo + bcast) 2 + lt 4 + acc 2 = 8
    with tc.tile_pool(name="pj_psum", bufs=2, space="PSUM") as pj_psum, \
         tc.tile_pool(name="lt_psum", bufs=2, space="PSUM") as lt_psum, \
         tc.tile_pool(name="acc_psum", bufs=2, space="PSUM") as acc_psum, \
         tc.tile_pool(name="xin", bufs=2) as xin_pool, \
         tc.tile_pool(name="xvin", bufs=3) as xvin_pool, \
         tc.tile_pool(name="pt", bufs=4) as pt_pool, \
         tc.tile_pool(name="outT", bufs=2) as outT_pool, \
         tc.tile_pool(name="nrm", bufs=4) as nrm_pool, \
         tc.tile_pool(name="osb", bufs=2) as out_pool:

        xq_tiles = {}

        _qps_box = {}

        def q_proj_unit(msb, cb, u):
            if u == 0:
                _qps_box[(msb, cb)] = pj_psum.tile(
                    [128, 512], F32, tag="ps", name=f"psq{msb}{cb}")
            ps = _qps_box[(msb, cb)]
            xq_t = xq_tiles[msb]
            for kc in range(4 * u, 4 * u + 4):
                nc.tensor.matmul(
                    ps[:],
                    wq_sb[:, kc * CPC + cb * 128: kc * CPC + (cb + 1) * 128],
                    xq_t[:, kc * 512:(kc + 1) * 512],
                    start=(kc == 0), stop=(kc == KC - 1),
                )
            if u == 3:
                nc.vector.tensor_scalar_add(
                    qT_sb[:, cb * S + msb * 512: cb * S + (msb + 1) * 512],
                    ps[:], bq_sb[:, cb:cb + 1],
                )

        def q_proj_cb(msb, cb):
            for u in range(4):
                q_proj_unit(msb, cb, u)

        # --- single-bank filler units for projections -------------------
        # Each owner (a K block, a V half-pass, a Q cb-pass) accumulates in
        # one pj bank across its consecutive units, so units from different
        # owners can interleave with attention j-steps without deadlocking
        # the 2-bank pj ring.
        _kps_box = {}
        xk_tiles = {}

        def k_proj_unit(nsb, u):
            if u == 0:
                _kps_box[nsb] = pj_psum.tile([128, 512], F32, tag="ps",
                                             name=f"psk{nsb}")
            ps = _kps_box[nsb]
            xk_t = xk_tiles[nsb]
            for kc in range(4 * u, 4 * u + 4):
                nc.tensor.matmul(
                    ps[:], wk_sb[:, kc * 128:(kc + 1) * 128],
                    xk_t[:, kc * 512:(kc + 1) * 512],
                    start=(kc == 0), stop=(kc == KC - 1),
                )
            if u == 3:
                nc.vector.tensor_scalar_add(
                    kT_sb[:, nsb * 512:(nsb + 1) * 512], ps[:], bk_sb[:]
                )

        _vps_box = {}
        xv_tiles = {}

        def v_proj_unit(nbp, half, u):
            if u == 0:
                _vps_box[(nbp, half)] = pj_psum.tile(
                    [128, 128], F32, tag="ps", name=f"psv{nbp}{half}")
            ps = _vps_box[(nbp, half)]
            xv_t = xv_tiles[nbp]
            for kc in range(4 * u, 4 * u + 4):
                nc.tensor.matmul(
                    ps[:], xv_t[:, kc * 256 + half * 128:
                                kc * 256 + half * 128 + 128],
                    wv_sb[:, kc * 128:(kc + 1) * 128],
                    start=(kc == 0), stop=(kc == KC - 1),
                )
            if u == 3:
                blk = 2 * nbp + half
                nc.vector.tensor_copy(
                    v_sb[:, blk * VW: blk * VW + 64], ps[:, 0:64])
                nc.vector.tensor_copy(
                    v_sb[:, blk * VW + 65: blk * VW + 129], ps[:, 64:128])

        def xk_dma(nsb):
            xk_t = xin_pool.tile([128, KC * 512], BF16, tag="xk",
                                 name=f"xk{nsb}")
            nc.sync.dma_start(
                xk_t[:].rearrange("p (kc n) -> p kc n", kc=KC),
                xkT.rearrange("(kc p) n -> p kc n", p=128)[
                    :, :, nsb * 512:(nsb + 1) * 512],
            )
            xk_tiles[nsb] = xk_t

        def xv_dma(nbp):
            xv_t = xvin_pool.tile([128, KC * 256], BF16, tag="xv",
                                  name=f"xv{nbp}")
            nc.sync.dma_start(
                xv_t[:].rearrange("p (kc n) -> p kc n", kc=KC),
                xvT.rearrange("(kc p) n -> p kc n", p=128)[
                    :, :, nbp * 256:(nbp + 1) * 256],
            )
            xv_tiles[nbp] = xv_t

        # --- phase A: the minimum needed before attention msb0 ----------
        # K block 0 (4 sub-DMAs so the PE starts ~5us in), V blocks 0:4,
        # Q msb0 — everything else streams just-in-time as filler work.
        def wk_dma(h):
            nc.sync.dma_start(
                wk_sb[:, h * 4 * 128:(h + 1) * 4 * 128],
                wk[:, h * 4 * 128:(h + 1) * 4 * 128],
            )

        with tc.tile_pool(name="xk0", bufs=4) as xk0_pool:
            xk0_t = [xk0_pool.tile([128, 4 * 512], BF16, tag="xk0",
                                   name=f"xk0{i}") for i in range(4)]
            for i in range(4):
                wk_dma(i)
                nc.sync.dma_start(
                    xk0_t[i][:].rearrange("p (kc n) -> p kc n", kc=4),
                    xkT.rearrange("(kc p) n -> p kc n", p=128)[
                        :, 4 * i: 4 * i + 4, 0:512],
                )
            nc.sync.dma_start(bk_sb[:], bk[:])
            nc.sync.dma_start(wv_sb[:], wv[:])
            # xv0 ahead of the wq/xq0 halves: V0 is the first PE work
            # after K0; the Q chunks land while V0 runs.
            xv_dma(0)
            nc.sync.dma_start(
                bq_sb[:].rearrange("p (cb o) -> p cb o", cb=4),
                bq.rearrange("(cb p) o -> p cb o", p=128),
            )
            nc.sync.dma_start(mask_sb[:], trimask[:])
            xq_t0 = xin_pool.tile([128, KC * 512], BF16, tag="xk", name="xq0")
            xq_tiles[0] = xq_t0
            for h in range(2):
                nc.sync.dma_start(
                    xq_t0[:, h * 8 * 512:(h + 1) * 8 * 512].rearrange(
                        "p (kc n) -> p kc n", kc=8),
                    xqT.rearrange("(kc p) n -> p kc n", p=128)[
                        :, h * 8:(h + 1) * 8, 0:512],
                )
                nc.sync.dma_start(
                    wq_sb[:, h * 8 * CPC:(h + 1) * 8 * CPC].rearrange(
                        "p (kc c) -> p kc c", kc=8),
                    wq.rearrange("(kc p) c -> p kc c", p=128)[
                        :, h * 8:(h + 1) * 8],
                )
                if h == 0:
                    xv_dma(1)
            ps = pj_psum.tile([128, 512], F32, tag="ps", name="psk0")
            for kc in range(KC):
                nc.tensor.matmul(
                    ps[:], wk_sb[:, kc * 128:(kc + 1) * 128],
                    xk0_t[kc // 4][:, (kc % 4) * 512:(kc % 4 + 1) * 512],
                    start=(kc == 0), stop=(kc == KC - 1),
                )
            nc.vector.tensor_scalar_add(kT_sb[:, 0:512], ps[:], bk_sb[:])

        for half in range(2):
            for u in range(4):
                v_proj_unit(0, half, u)
        q_proj_cb(0, 0)
        for half in range(2):
            for u in range(4):
                v_proj_unit(1, half, u)
        q_proj_cb(0, 1)

        # Wo for one m-superblock as 16 filler units (one per mb/db pso
        # group); the out-row DMA rides the db==3 unit.
        def wo_group(msb, outT_t, o_t_box, mb, db):
            if db == 0:
                o_t_box[mb] = out_pool.tile([128, 2048], BF16, tag="osb",
                                            name=f"osb{msb}{mb}")
            o_t = o_t_box[mb]
            pso = pj_psum.tile([128, 512], F32, tag="ps",
                               name=f"pso{msb}{mb}{db}")
            for cb in range(4):
                nc.tensor.matmul(
                    pso[:],
                    outT_t[:, cb * 512 + mb * 128: cb * 512 + (mb + 1) * 128],
                    wo_sb[:, cb * D + db * 512: cb * D + (db + 1) * 512],
                    start=(cb == 0), stop=(cb == 3),
                )
            nc.vector.tensor_copy(o_t[:, db * 512:(db + 1) * 512], pso[:])
            if msb == 3 and mb == 3:
                nc.sync.dma_start(
                    out[msb * 512 + mb * 128: msb * 512 + (mb + 1) * 128,
                        db * 512:(db + 1) * 512],
                    o_t[:, db * 512:(db + 1) * 512],
                )
            elif db == 3:
                nc.sync.dma_start(
                    out[msb * 512 + mb * 128: msb * 512 + (mb + 1) * 128, :],
                    o_t[:],
                )

        # Per m-superblock: attention with filler work (Wo groups of the
        # previous superblock + Q-projection passes of the next) emitted
        # between j-steps so the PE has backlog whenever the attnv chain
        # waits on the scalar engine's exp.
        pending = []   # filler closures, emitted oldest-first
        for msb in range(MSB):
            if msb == 0:
                # Q0 cb2/cb3 ride as fillers inside attn0 p0/p1 (attention
                # p reads only its own cb's qT slice, so p2/p3's data is
                # ready just in time and attn0 starts ~8us earlier).
                for cb in (2, 3):
                    for qu in range(4):
                        pending.append(
                            (lambda c, u: lambda: q_proj_unit(0, c, u))(
                                cb, qu))
            if msb + 1 < MSB:
                xk_dma(msb + 1)
                xv_dma(2 * msb + 2)
                xv_dma(2 * msb + 3)
                if msb == 0:
                    nc.sync.dma_start(
                        wo_sb[:].rearrange("p (cb d) -> p cb d", cb=4),
                        wo.rearrange("(cb p) d -> p cb d", p=128),
                    )
                xq_t = xin_pool.tile([128, KC * 512], BF16, tag="xk",
                                     name=f"xq{msb + 1}")
                nc.sync.dma_start(
                    xq_t[:].rearrange("p (kc n) -> p kc n", kc=KC),
                    xqT.rearrange("(kc p) n -> p kc n", p=128)[
                        :, :, (msb + 1) * 512:(msb + 2) * 512],
                )
                xq_tiles[msb + 1] = xq_t
                for u in range(4):
                    pending.append(
                        (lambda n, uu: lambda: k_proj_unit(n, uu))(msb + 1, u))
                for nbp in (2 * msb + 2, 2 * msb + 3):
                    for half in range(2):
                        for u in range(4):
                            pending.append(
                                (lambda n, hh, uu: lambda:
                                 v_proj_unit(n, hh, uu))(nbp, half, u))
                for cb in range(4):
                    for qu in range(4):
                        pending.append(
                            (lambda m, c, u: lambda: q_proj_unit(m, c, u))(
                                msb + 1, cb, qu))

            n_steps = 4 * (4 * msb + 4)
            n_fill = len(pending)
            step = 0
            emitted = 0

            outT_t = outT_pool.tile([128, 2048], BF16, tag="outT")
            for p in range(4):
                # acc A/B rows 0:64 = half out, row 64 = denominator;
                # the half-b eviction shifts partitions 0:64 -> 64:128.
                accA = acc_psum.tile([128, 512], F32, tag="acc",
                                     name=f"accA{msb}{p}")
                accB = acc_psum.tile([128, 512], F32, tag="acc",
                                     name=f"accB{msb}{p}")
                njb = 4 * msb + 4
                for j in range(njb):
                    if j < 4 * msb:
                        moff, W = 0, 512
                    else:
                        t = j - 4 * msb
                        moff, W = 128 * t, 512 - 128 * t
                    first = (j == 0)
                    last = (j == njb - 1)
                    qlo = qT_sb[0:64, p * S + msb * 512 + moff:
                                p * S + msb * 512 + moff + W]
                    qhi = qT_sb[64:128, p * S + msb * 512 + moff:
                                p * S + msb * 512 + moff + W]
                    lt = lt_psum.tile([128, 1024], F32, tag="lt")
                    nc.tensor.matmul(
                        lt[:, 0:W],
                        kT_sb[0:64, j * 128:(j + 1) * 128], qlo,
                        start=True, stop=True, tile_position=(0, 0),
                    )
                    nc.tensor.matmul(
                        lt[:, 512:512 + W],
                        kT_sb[64:128, j * 128:(j + 1) * 128], qhi,
                        start=True, stop=True, tile_position=(64, 0),
                    )
                    pt = pt_pool.tile([128, 1024], BF16, tag="pt")
                    if W == 512:
                        nc.scalar.activation(pt[:], lt[:], Exp,
                                             scale=INV_SQRT_DIMK)
                    else:
                        # one strided activation covers both halves
                        nc.scalar.activation(
                            pt[:].rearrange("p (h w) -> p h w", h=2)[:, :, 0:W],
                            lt[:].rearrange("p (h w) -> p h w", h=2)[:, :, 0:W],
                            Exp, scale=INV_SQRT_DIMK)
                    if j >= 4 * msb:  # diagonal: mask the leading triangle
                        ptm = pt[:].rearrange("p (h w) -> p h w",
                                              h=2)[:, :, 0:128]
                        nc.vector.tensor_mul(
                            ptm, ptm,
                            mask_sb[:].rearrange("p (h w) -> p h w", h=2))
                    # attn @ v, 65-wide weights per half ([v | 1]): the
                    # denominator lands in psum row 64 of each acc bank.
                    nc.tensor.matmul(
                        accA[0:65, moff:moff + W],
                        v_sb[:, j * VW: j * VW + 65], pt[:, 0:W],
                        start=first, stop=last, tile_position=(0, 0),
                    )
                    nc.tensor.matmul(
                        accB[0:65, moff:moff + W],
                        v_sb[:, j * VW + 65: j * VW + VW], pt[:, 512:512 + W],
                        start=first, stop=last, tile_position=(0, 0),
                    )
                    step += 1
                    want = n_fill * step // n_steps
                    while pending and emitted < want:
                        pending.pop(0)()
                        emitted += 1
                # normalize: reciprocal rows -> PE outer-product broadcast
                # -> psum-eviction multiplies (with partition-window shift
                # for half b).
                recip_a = nrm_pool.tile([1, 512], BF16, tag="recipa",
                                        name=f"rca{msb}{p}")
                recip_b = nrm_pool.tile([1, 512], BF16, tag="recipb",
                                        name=f"rcb{msb}{p}")
                bcast = nrm_pool.tile([128, 512], F32, tag="bc",
                                      name=f"bc{msb}{p}")
                bcast_ps = pj_psum.tile([128, 512], F32, tag="ps",
                                        name=f"bcp{msb}{p}")
                with nc.allow_low_precision(reason="1/den bf16: 0.4% "
                                            "normalization err, tol 2e-2"):
                    nc.vector.reciprocal(recip_a[:], accA[64:65, :])
                    nc.vector.reciprocal(recip_b[:], accB[64:65, :])
                nc.tensor.matmul(
                    bcast_ps[0:64, :], ones_row[:], recip_a[:],
                    start=True, stop=True, tile_position=(0, 0),
                )
                nc.tensor.matmul(
                    bcast_ps[64:128, :], ones_row[:], recip_b[:],
                    start=True, stop=True, tile_position=(0, 64),
                )
                nc.vector.tensor_copy(bcast[:], bcast_ps[:])
                nc.vector.tensor_mul(
                    outT_t[0:64, p * 512:(p + 1) * 512],
                    accA[0:64, :], bcast[0:64, :],
                )
                nc.vector.tensor_mul(
                    outT_t[64:128, p * 512:(p + 1) * 512],
                    accB[0:64, :], bcast[64:128, :],
                )

            # Drain any unissued fillers, then queue this superblock's
            # output projection as fillers for the next one.
            for f in pending:
                f()
            pending = []
            o_t_box = {}
            for mb in range(4):
                for db in range(4):
                    pending.append(
                        (lambda m, ot, ob, a, b: lambda: wo_group(m, ot, ob, a, b))(
                            msb, outT_t, o_t_box, mb, db))
        for f in pending:
            f()


_NC_CACHE = {}


def get_nc():
    if "nc" not in _NC_CACHE:
        _NC_CACHE["nc"] = build_bass()
    return _NC_CACHE["nc"]


def kernel(inputs_q, inputs_k, inputs_v, Wq, bq, Wk, bk, Wv, bv, Wo, bo):
    inputs_q = np.asarray(inputs_q, np.float32)
    inputs_k = np.asarray(inputs_k, np.float32)
    inputs_v = np.asarray(inputs_v, np.float32)
    Wq = np.asarray(Wq, np.float32)
    Wk = np.asarray(Wk, np.float32)
    Wv = np.asarray(Wv, np.float32)
    Wo = np.asarray(Wo, np.float32)
    bq = np.asarray(bq, np.float32)
    bk = np.asarray(bk, np.float32)
    bv = np.asarray(bv, np.float32)
    bo = np.asarray(bo, np.float32)

    nc = get_nc()
    trimask = np.tile(np.triu(np.ones((128, 128), NPBF16)), (1, 2))

    xT = {}
    for b in range(B):
        xT[("q", b)] = np.ascontiguousarray(inputs_q[b].T.astype(NPBF16))
        xT[("k", b)] = np.ascontiguousarray(inputs_k[b].T.astype(NPBF16))
        xT[("v", b)] = np.ascontiguousarray(inputs_v[b].T.astype(NPBF16))

    in_maps = []
    for c in range(8):
        b = c // 4
        g0 = 2 * (c % 4)
        g1 = g0 + 1
        # pair-major channel permutation: (head p of g0, head p of g1), p=0..3
        perm = []
        for p in range(HPG):
            perm.extend(range(256 * g0 + 64 * p, 256 * g0 + 64 * p + 64))
            perm.extend(range(256 * g1 + 64 * p, 256 * g1 + 64 * p + 64))
        perm = np.array(perm)
        in_maps.append({
            "xqT": xT[("q", b)],
            "xkT": xT[("k", b)],
            "xvT": xT[("v", b)],
            "wq": np.ascontiguousarray(Wq[:, perm].astype(NPBF16)),
            "wk": np.ascontiguousarray(
                Wk[:, 64 * g0: 64 * g0 + 128].astype(NPBF16)
                .reshape(KC, 128, 128).transpose(1, 0, 2).reshape(128, -1)),
            "wv": np.ascontiguousarray(
                Wv[:, 64 * g0: 64 * g0 + 128].astype(NPBF16)
                .reshape(KC, 128, 128).transpose(1, 0, 2).reshape(128, -1)),
            "wo": np.ascontiguousarray(Wo[perm, :].astype(NPBF16)),
            "bq": np.ascontiguousarray(bq[perm].reshape(CPC, 1)),
            "bk": np.ascontiguousarray(bk[64 * g0: 64 * g0 + 128].reshape(128, 1)),
            "trimask": trimask,
        })

    res = run_bass_kernel_spmd(nc, in_maps, list(range(8)))

    # bv passes through (attention rows sum to 1): out += bv_expand @ Wo + bo
    bv_expand = np.repeat(bv.reshape(NKV, 1, HD), HPG, axis=1).reshape(D)
    corr = (bv_expand.astype(np.float64) @ Wo.astype(np.float64)) + bo

    outp = np.zeros((B, S, D), np.float64)
    for c in range(8):
        outp[c // 4] += res.results[c]["out"].astype(np.float64)
    outp += corr
    return outp.astype(np.float32)
